# revision 7
# baseline (speedup 1.0000x reference)
"""Trainium2 Bass kernel for the CatRSDNet 5-model MC-dropout LSTM ensemble.

Problem: X [8192,1,1664] -> 5 independent LSTM(1664->128) replicas with
variational dropout masks folded into the weights, 8192 serial timesteps,
then 4 small linear heads ensemble-averaged over the 5 replicas.

Strategy (8 NeuronCores, SPMD, one NEFF):
  - model-parallel: core r simulates ensemble member r (cores 5-7 duplicate
    model 4; their outputs are ignored).
  - Phase A (per core): gx[t] = W_ih_m @ x[t] + b  for all t as large
    matmuls (X transposed on host so F sits on partitions), streamed to a
    DRAM scratch buffer.
  - Phase B: the serial recurrence.  Per step: 4 accumulating 128x128
    matvecs (W_hh.T stationary, h moving) on top of gx (injected into PSUM
    via an identity matmul, amortized over groups of 4 steps), one Sigmoid
    over all 4 gates (tanh(g) computed as 2*sigmoid(2g)-1 with the g-block
    weights pre-doubled on the host), two fused tensor_scalar ops for the
    cell update (carrying z = c/2), one Tanh, one multiply for h.
  - Heads: one [128,15] matmul per 512-step chunk over the stored Y.
Host folds dropout masks + biases into weights, sums per-core head outputs.
"""

import numpy as np

T_LEN = 8192
FEAT = 1664
HID = 128
NM = 5
G4 = 4 * HID  # 512 gates
NHEAD = 15  # 11 + 2 + 1 + 1
KC = FEAT // 128  # 13 contraction tiles for phase A
GROUP = 4  # steps sharing one PSUM gate tile / one gx-inject matmul


def _build(T=T_LEN, whh_dtype="float32"):
    """Build the Bass program. Returns nc."""
    from contextlib import ExitStack

    import concourse.bass as bass
    import concourse.tile as tile
    from concourse import bacc, mybir
    from concourse.bass import ts

    f32 = mybir.dt.float32
    wdt = getattr(mybir.dt, whh_dtype)
    chunk = min(512, T)
    nch = T // chunk
    assert T % chunk == 0 and chunk % GROUP == 0

    nc = bacc.Bacc(
        "TRN2", target_bir_lowering=False, debug=False, num_devices=8
    )
    XT = nc.dram_tensor("XT", [FEAT, T], f32, kind="ExternalInput")
    WIH = nc.dram_tensor("WIH", [FEAT, G4], f32, kind="ExternalInput")
    BB = nc.dram_tensor("BB", [1, G4], f32, kind="ExternalInput")
    WHH = nc.dram_tensor("WHH", [HID, G4], wdt, kind="ExternalInput")
    WCT = nc.dram_tensor("WCT", [HID, NHEAD], f32, kind="ExternalInput")
    IDT = nc.dram_tensor("IDT", [HID, HID], f32, kind="ExternalInput")
    Z = nc.dram_tensor("Z", [NHEAD, T], f32, kind="ExternalOutput")

    sig = mybir.ActivationFunctionType.Sigmoid
    tanh = mybir.ActivationFunctionType.Tanh
    mult = mybir.AluOpType.mult
    add = mybir.AluOpType.add
    sub = mybir.AluOpType.subtract

    with tile.TileContext(nc) as tc, ExitStack() as ctx:
        const = ctx.enter_context(tc.tile_pool(name="const", bufs=1))
        dram = ctx.enter_context(tc.tile_pool(name="dram", bufs=1, space="DRAM"))
        xtp = ctx.enter_context(tc.tile_pool(name="xtp", bufs=2))
        gxps = ctx.enter_context(tc.tile_pool(name="gxps", bufs=2, space="PSUM"))
        gxsb = ctx.enter_context(tc.tile_pool(name="gxsb", bufs=2))
        hsb = ctx.enter_context(tc.tile_pool(name="hsb", bufs=2))
        gxcp = ctx.enter_context(tc.tile_pool(name="gxcp", bufs=2))
        gps = ctx.enter_context(tc.tile_pool(name="gps", bufs=3, space="PSUM"))
        hps = ctx.enter_context(tc.tile_pool(name="hps", bufs=2, space="PSUM"))
        ypool = ctx.enter_context(tc.tile_pool(name="ypool", bufs=1))
        sp = ctx.enter_context(tc.tile_pool(name="sp", bufs=4))
        wp = ctx.enter_context(tc.tile_pool(name="wp", bufs=4))
        zp = ctx.enter_context(tc.tile_pool(name="zp", bufs=4))
        tp = ctx.enter_context(tc.tile_pool(name="tp", bufs=4))

        # ---- constants ----
        whh_sb = const.tile([HID, G4], wdt)
        nc.sync.dma_start(whh_sb[:], WHH[:])
        idt_sb = const.tile([HID, HID], f32)
        nc.sync.dma_start(idt_sb[:], IDT[:])
        wct_sb = const.tile([HID, NHEAD], f32)
        nc.sync.dma_start(wct_sb[:], WCT[:])
        bb_sb = const.tile([1, G4], f32)
        nc.sync.dma_start(bb_sb[:], BB[:])
        wih_sb = const.tile([128, KC, G4], f32)
        nc.sync.dma_start(wih_sb[:], WIH.rearrange("(k p) g -> p k g", p=128)[:])
        ones_sb = const.tile([1, chunk], f32)
        nc.vector.memset(ones_sb[:], 1.0)
        zero_f32 = const.tile([HID, 1], f32)
        nc.vector.memset(zero_f32[:], 0.0)
        if wdt != f32:
            zero_w = const.tile([HID, 1], wdt)
            nc.vector.memset(zero_w[:], 0.0)
        else:
            zero_w = zero_f32

        GX = dram.tile([4, HID, T], f32)
        gx_rd = GX.rearrange("j p t -> p j t")

        def emit_phase_a(n):
            xt = xtp.tile([128, KC, chunk], f32, tag="xt")
            nc.sync.dma_start(
                xt[:], XT.rearrange("(k p) t -> p k t", p=128)[:, :, ts(n, chunk)]
            )
            for j in range(4):
                ps = gxps.tile([128, chunk], f32, tag="gx")
                for k in range(KC):
                    nc.tensor.matmul(
                        ps[:],
                        lhsT=wih_sb[:, k, ts(j, 128)],
                        rhs=xt[:, k, :],
                        start=(k == 0),
                        stop=False,
                    )
                nc.tensor.matmul(
                    ps[:],
                    lhsT=bb_sb[:, ts(j, 128)],
                    rhs=ones_sb[:],
                    start=False,
                    stop=True,
                )
                st = gxsb.tile([128, chunk], f32, tag="gxst")
                nc.vector.tensor_copy(st[:], ps[:])
                nc.sync.dma_start(GX[j, :, ts(n, chunk)], st[:])

        Y = ypool.tile([HID, T], wdt)
        state = {"z": zero_f32, "h": zero_w}

        def emit_phase_b(n):
            gxc = gxcp.tile([128, 4, chunk], f32, tag="gxc")
            nc.sync.dma_start(gxc[:], gx_rd[:, :, ts(n, chunk)])
            for g0 in range(0, chunk, GROUP):
                ps = gps.tile([128, 4 * GROUP], f32, tag="gates")
                # inject gx for GROUP steps: out[:, (t,j)] laid t-major
                nc.tensor.matmul(
                    ps.rearrange("p (t j) -> p j t", j=4)[:],
                    lhsT=idt_sb[:],
                    rhs=gxc[:, :, g0 : g0 + GROUP],
                    start=True,
                    stop=False,
                    skip_group_check=True,
                )
                for gl in range(GROUP):
                    t = n * chunk + g0 + gl
                    col = 4 * gl
                    h_prev = state["h"]
                    for j in range(4):
                        nc.tensor.matmul(
                            ps[:, col + j : col + j + 1],
                            lhsT=whh_sb[:, ts(j, 128)],
                            rhs=h_prev[:],
                            start=False,
                            stop=(gl == GROUP - 1 and j == 3),
                            skip_group_check=True,
                        )
                    s = sp.tile([HID, 4], f32, tag="s")
                    nc.scalar.activation(s[:], ps[:, col : col + 4], sig)
                    w = wp.tile([HID, 1], f32, tag="w")
                    nc.vector.tensor_scalar(
                        w[:], s[:, 2:3], 0.5, s[:, 0:1], op0=sub, op1=mult
                    )
                    znew = zp.tile([HID, 1], f32, tag="z")
                    nc.vector.tensor_scalar(
                        znew[:], state["z"][:], s[:, 1:2], w[:], op0=mult, op1=add
                    )
                    tcv = tp.tile([HID, 1], f32, tag="tc")
                    nc.scalar.activation(tcv[:], znew[:], tanh, scale=2.0)
                    nc.vector.tensor_scalar(
                        Y[:, t : t + 1], tcv[:], s[:, 3:4], None, op0=mult
                    )
                    state["z"] = znew
                    state["h"] = Y[:, t : t + 1]

        def emit_heads(n):
            zps = hps.tile([NHEAD, chunk], f32, tag="head")
            nc.tensor.matmul(
                zps[:], lhsT=wct_sb[:], rhs=Y[:, ts(n, chunk)], start=True, stop=True
            )
            zst = hsb.tile([NHEAD, chunk], f32, tag="zst")
            nc.vector.tensor_copy(zst[:], zps[:])
            nc.sync.dma_start(Z[:, ts(n, chunk)], zst[:])

        # software-pipelined emission: phase A chunk n+1 overlaps phase B chunk n
        emit_phase_a(0)
        for n in range(nch):
            if n + 1 < nch:
                emit_phase_a(n + 1)
            emit_phase_b(n)
            emit_heads(n)

    nc.compile()
    return nc


def _prep_inputs(inputs, whh_np=np.float32):
    """Host-side folding of masks/biases into weights. Returns per-model maps."""
    X = np.asarray(inputs["X"], np.float32)[:, 0, :]  # [T, F]
    T = X.shape[0]
    XT = np.ascontiguousarray(X.T)  # [F, T]
    IDT = np.eye(HID, dtype=np.float32)

    W_ih = np.asarray(inputs["W_ih"], np.float32)
    W_hh = np.asarray(inputs["W_hh"], np.float32)
    b_ih = np.asarray(inputs["b_ih"], np.float32)
    b_hh = np.asarray(inputs["b_hh"], np.float32)
    mask_x = np.asarray(inputs["mask_x"], np.float32)
    mask_h = np.asarray(inputs["mask_h"], np.float32)
    heads_w = [np.asarray(inputs[k], np.float32) for k in ("W1", "W2", "W3", "W4")]
    heads_b = [np.asarray(inputs[k], np.float32) for k in ("b1", "b2", "b3", "b4")]

    per_model = []
    for r in range(NM):
        wih = W_ih[r] * mask_x[r][None, :]
        whh = W_hh[r] * mask_h[r][None, :]
        bt = b_ih[r] + b_hh[r]
        wih = wih.copy()
        whh = whh.copy()
        bt = bt.copy()
        wih[2 * HID : 3 * HID] *= 2.0
        whh[2 * HID : 3 * HID] *= 2.0
        bt[2 * HID : 3 * HID] *= 2.0
        wc = np.concatenate([w[r] for w in heads_w], axis=0) / NM  # [15, 128]
        per_model.append(
            {
                "XT": XT,
                "WIH": np.ascontiguousarray(wih.T),
                "BB": np.ascontiguousarray(bt[None, :]),
                "WHH": np.ascontiguousarray(whh.T).astype(whh_np),
                "WCT": np.ascontiguousarray(wc.T),
                "IDT": IDT,
            }
        )
    bias_mean = np.concatenate([b.mean(axis=0) for b in heads_b])  # [15]
    return per_model, bias_mean, T


_CACHE = {}


def _run(inputs, T, whh_dtype="float32", trace=False, n_cores=8):
    from concourse.bass_utils import run_bass_kernel_spmd

    whh_np = np.float32 if whh_dtype == "float32" else None
    if whh_np is None:
        import ml_dtypes

        whh_np = ml_dtypes.bfloat16
    per_model, bias_mean, T_in = _prep_inputs(inputs, whh_np)
    assert T_in == T
    key = (T, whh_dtype)
    if key not in _CACHE:
        _CACHE[key] = _build(T, whh_dtype)
    nc = _CACHE[key]
    in_maps = [per_model[min(r, NM - 1)] for r in range(n_cores)]
    res = run_bass_kernel_spmd(nc, in_maps, core_ids=list(range(n_cores)), trace=trace)
    Zsum = np.zeros((NHEAD, T), np.float32)
    for r in range(NM):
        Zsum += res.results[r]["Z"]
    out = Zsum + bias_mean[:, None]
    step_logits = np.ascontiguousarray(out[0:11].T)
    experience = np.ascontiguousarray(out[11:13].T)
    rsd = np.ascontiguousarray(out[13:14].T)
    s = np.ascontiguousarray(out[14:15].T)
    return (step_logits, experience, rsd, s), res


def kernel(**inputs):
    outs, _ = _run(inputs, T_LEN, whh_dtype="float32")
    return outs


# revision 8
# speedup vs baseline: 1.8987x; 1.8987x over previous
"""Trainium2 Bass kernel for the CatRSDNet 5-model MC-dropout LSTM ensemble.

Problem: X [8192,1,1664] -> 5 independent LSTM(1664->128) replicas with
variational dropout masks folded into the weights, 8192 serial timesteps,
then 4 small linear heads ensemble-averaged over the 5 replicas.

Strategy (8 NeuronCores, SPMD, one NEFF):
  - model-parallel: core r simulates ensemble member r (cores 5-7 duplicate
    model 4; their outputs are ignored).
  - Phase A (per core): gx[t] = W_ih_m @ x[t] + b  for all t as large
    matmuls (X transposed on host so F sits on partitions), streamed to a
    DRAM scratch buffer.  fp32 inputs are split into bf16 hi+lo pairs and
    multiplied 3-term (HiHi + HiLo + LoHi) so the PE runs at bf16 rate with
    ~fp24 precision; gx is stored to DRAM as a bf16 hi/lo pair.
  - Phase B: the serial recurrence.  Per step: 4 accumulating 128x128 bf16
    matvecs (W_hh.T stationary, h moving) on top of gx (injected into PSUM
    via identity matmuls, amortized over groups of 4 steps), one Sigmoid
    over all 4 gates (tanh(g) computed as 2*sigmoid(2g)-1 with the g-block
    weights pre-doubled on the host), two fused tensor_scalar ops for the
    cell update (carrying z = c/2 in fp32), one Tanh, one multiply for h.
  - Heads: one [128,15] matmul per 512-step chunk over the stored Y.
Host folds dropout masks + biases into weights, sums per-core head outputs.
"""

import numpy as np

T_LEN = 8192
FEAT = 1664
HID = 128
NM = 5
G4 = 4 * HID  # 512 gates
NHEAD = 15  # 11 + 2 + 1 + 1
KC = FEAT // 128  # 13 contraction tiles for phase A
GROUP = 4  # steps sharing one PSUM gate tile / one gx-inject matmul


def _build(T=T_LEN, mode="bf16"):
    """Build the Bass program. mode: 'bf16' (fast) or 'fp32' (fallback)."""
    from contextlib import ExitStack

    import concourse.bass as bass
    import concourse.tile as tile
    from concourse import bacc, mybir
    from concourse.bass import ts

    f32 = mybir.dt.float32
    bf16 = mybir.dt.bfloat16
    wdt = bf16 if mode == "bf16" else f32
    chunk = min(512, T)
    nch = T // chunk
    assert T % chunk == 0 and chunk % GROUP == 0

    nc = bacc.Bacc("TRN2", target_bir_lowering=False, debug=False, num_devices=8)
    if mode == "bf16":
        XT_HI = nc.dram_tensor("XT_HI", [FEAT, T], bf16, kind="ExternalInput")
        XT_LO = nc.dram_tensor("XT_LO", [FEAT, T], bf16, kind="ExternalInput")
        WIH_HI = nc.dram_tensor("WIH_HI", [FEAT, G4], bf16, kind="ExternalInput")
        WIH_LO = nc.dram_tensor("WIH_LO", [FEAT, G4], bf16, kind="ExternalInput")
        BB_HI = nc.dram_tensor("BB_HI", [1, G4], bf16, kind="ExternalInput")
        BB_LO = nc.dram_tensor("BB_LO", [1, G4], bf16, kind="ExternalInput")
    else:
        XT = nc.dram_tensor("XT", [FEAT, T], f32, kind="ExternalInput")
        WIH = nc.dram_tensor("WIH", [FEAT, G4], f32, kind="ExternalInput")
        BB = nc.dram_tensor("BB", [1, G4], f32, kind="ExternalInput")
    WHH = nc.dram_tensor("WHH", [HID, G4], wdt, kind="ExternalInput")
    WCT = nc.dram_tensor("WCT", [HID, NHEAD], wdt, kind="ExternalInput")
    IDT = nc.dram_tensor("IDT", [HID, HID], wdt, kind="ExternalInput")
    Z = nc.dram_tensor("Z", [NHEAD, T], f32, kind="ExternalOutput")

    sig = mybir.ActivationFunctionType.Sigmoid
    tanh = mybir.ActivationFunctionType.Tanh
    mult = mybir.AluOpType.mult
    add = mybir.AluOpType.add
    sub = mybir.AluOpType.subtract

    with tile.TileContext(nc) as tc, ExitStack() as ctx:
        const = ctx.enter_context(tc.tile_pool(name="const", bufs=1))
        dram = ctx.enter_context(tc.tile_pool(name="dram", bufs=1, space="DRAM"))
        xtp = ctx.enter_context(tc.tile_pool(name="xtp", bufs=2))
        gxps = ctx.enter_context(tc.tile_pool(name="gxps", bufs=2, space="PSUM"))
        gxsb = ctx.enter_context(tc.tile_pool(name="gxsb", bufs=2))
        hsb = ctx.enter_context(tc.tile_pool(name="hsb", bufs=2))
        gxcp = ctx.enter_context(tc.tile_pool(name="gxcp", bufs=2))
        gps = ctx.enter_context(tc.tile_pool(name="gps", bufs=3, space="PSUM"))
        hps = ctx.enter_context(tc.tile_pool(name="hps", bufs=2, space="PSUM"))
        ypool = ctx.enter_context(tc.tile_pool(name="ypool", bufs=1))
        sp = ctx.enter_context(tc.tile_pool(name="sp", bufs=4))
        wp = ctx.enter_context(tc.tile_pool(name="wp", bufs=4))
        zp = ctx.enter_context(tc.tile_pool(name="zp", bufs=4))
        tp = ctx.enter_context(tc.tile_pool(name="tp", bufs=4))

        # ---- constants ----
        whh_sb = const.tile([HID, G4], wdt)
        nc.sync.dma_start(whh_sb[:], WHH[:])
        idt_sb = const.tile([HID, HID], wdt)
        nc.sync.dma_start(idt_sb[:], IDT[:])
        wct_sb = const.tile([HID, NHEAD], wdt)
        nc.sync.dma_start(wct_sb[:], WCT[:])
        if mode == "bf16":
            bbh_sb = const.tile([1, G4], bf16)
            nc.sync.dma_start(bbh_sb[:], BB_HI[:])
            bbl_sb = const.tile([1, G4], bf16)
            nc.sync.dma_start(bbl_sb[:], BB_LO[:])
            wihh_sb = const.tile([128, KC, G4], bf16)
            nc.sync.dma_start(wihh_sb[:], WIH_HI.rearrange("(k p) g -> p k g", p=128)[:])
            wihl_sb = const.tile([128, KC, G4], bf16)
            nc.sync.dma_start(wihl_sb[:], WIH_LO.rearrange("(k p) g -> p k g", p=128)[:])
        else:
            bb_sb = const.tile([1, G4], f32)
            nc.sync.dma_start(bb_sb[:], BB[:])
            wih_sb = const.tile([128, KC, G4], f32)
            nc.sync.dma_start(wih_sb[:], WIH.rearrange("(k p) g -> p k g", p=128)[:])
        ones_sb = const.tile([1, chunk], wdt)
        nc.vector.memset(ones_sb[:], 1.0)
        zero_f32 = const.tile([HID, 1], f32)
        nc.vector.memset(zero_f32[:], 0.0)
        if wdt != f32:
            zero_w = const.tile([HID, 1], wdt)
            nc.vector.memset(zero_w[:], 0.0)
        else:
            zero_w = zero_f32

        if mode == "bf16":
            GXH = dram.tile([4, HID, T], bf16)
            GXL = dram.tile([4, HID, T], bf16)
            gxh_rd = GXH.rearrange("j p t -> p j t")
            gxl_rd = GXL.rearrange("j p t -> p j t")
        else:
            GX = dram.tile([4, HID, T], f32)
            gx_rd = GX.rearrange("j p t -> p j t")

        def emit_phase_a(n):
            if mode == "bf16":
                xth = xtp.tile([128, KC, chunk], bf16, tag="xth")
                nc.sync.dma_start(
                    xth[:],
                    XT_HI.rearrange("(k p) t -> p k t", p=128)[:, :, ts(n, chunk)],
                )
                xtl = xtp.tile([128, KC, chunk], bf16, tag="xtl")
                nc.sync.dma_start(
                    xtl[:],
                    XT_LO.rearrange("(k p) t -> p k t", p=128)[:, :, ts(n, chunk)],
                )
                for j in range(4):
                    ps = gxps.tile([128, chunk], f32, tag="gx")
                    for k in range(KC):
                        for wt, xt_ in (
                            (wihh_sb, xth),
                            (wihh_sb, xtl),
                            (wihl_sb, xth),
                        ):
                            nc.tensor.matmul(
                                ps[:],
                                lhsT=wt[:, k, ts(j, 128)],
                                rhs=xt_[:, k, :],
                                start=(k == 0 and wt is wihh_sb and xt_ is xth),
                                stop=False,
                                skip_group_check=True,
                            )
                    nc.tensor.matmul(
                        ps[:],
                        lhsT=bbh_sb[:, ts(j, 128)],
                        rhs=ones_sb[:],
                        start=False,
                        stop=False,
                        skip_group_check=True,
                    )
                    nc.tensor.matmul(
                        ps[:],
                        lhsT=bbl_sb[:, ts(j, 128)],
                        rhs=ones_sb[:],
                        start=False,
                        stop=True,
                        skip_group_check=True,
                    )
                    sth = gxsb.tile([128, chunk], bf16, tag="sth")
                    nc.vector.tensor_copy(sth[:], ps[:])
                    stw = gxsb.tile([128, chunk], f32, tag="stw")
                    nc.vector.tensor_copy(stw[:], sth[:])
                    stl = gxsb.tile([128, chunk], bf16, tag="stl")
                    nc.vector.tensor_tensor(
                        stl[:], ps[:], stw[:], op=sub
                    )
                    nc.sync.dma_start(GXH[j, :, ts(n, chunk)], sth[:])
                    nc.sync.dma_start(GXL[j, :, ts(n, chunk)], stl[:])
            else:
                xt = xtp.tile([128, KC, chunk], f32, tag="xt")
                nc.sync.dma_start(
                    xt[:], XT.rearrange("(k p) t -> p k t", p=128)[:, :, ts(n, chunk)]
                )
                for j in range(4):
                    ps = gxps.tile([128, chunk], f32, tag="gx")
                    for k in range(KC):
                        nc.tensor.matmul(
                            ps[:],
                            lhsT=wih_sb[:, k, ts(j, 128)],
                            rhs=xt[:, k, :],
                            start=(k == 0),
                            stop=False,
                        )
                    nc.tensor.matmul(
                        ps[:],
                        lhsT=bb_sb[:, ts(j, 128)],
                        rhs=ones_sb[:],
                        start=False,
                        stop=True,
                    )
                    st = gxsb.tile([128, chunk], f32, tag="gxst")
                    nc.vector.tensor_copy(st[:], ps[:])
                    nc.sync.dma_start(GX[j, :, ts(n, chunk)], st[:])

        Y = ypool.tile([HID, T], wdt)
        state = {"z": zero_f32, "h": zero_w}

        def emit_phase_b(n):
            if mode == "bf16":
                gxch = gxcp.tile([128, 4, chunk], bf16, tag="gxch")
                nc.sync.dma_start(gxch[:], gxh_rd[:, :, ts(n, chunk)])
                gxcl = gxcp.tile([128, 4, chunk], bf16, tag="gxcl")
                nc.sync.dma_start(gxcl[:], gxl_rd[:, :, ts(n, chunk)])
                injections = ((gxch, True), (gxcl, False))
            else:
                gxc = gxcp.tile([128, 4, chunk], f32, tag="gxc")
                nc.sync.dma_start(gxc[:], gx_rd[:, :, ts(n, chunk)])
                injections = ((gxc, True),)
            for g0 in range(0, chunk, GROUP):
                ps = gps.tile([128, 4 * GROUP], f32, tag="gates")
                # inject gx for GROUP steps: psum cols laid t-major (t, j)
                for src, is_first in injections:
                    nc.tensor.matmul(
                        ps.rearrange("p (t j) -> p j t", j=4)[:],
                        lhsT=idt_sb[:],
                        rhs=src[:, :, g0 : g0 + GROUP],
                        start=is_first,
                        stop=False,
                        skip_group_check=True,
                    )
                for gl in range(GROUP):
                    t = n * chunk + g0 + gl
                    col = 4 * gl
                    h_prev = state["h"]
                    for j in range(4):
                        nc.tensor.matmul(
                            ps[:, col + j : col + j + 1],
                            lhsT=whh_sb[:, ts(j, 128)],
                            rhs=h_prev[:],
                            start=False,
                            stop=(gl == GROUP - 1 and j == 3),
                            skip_group_check=True,
                        )
                    s = sp.tile([HID, 4], f32, tag="s")
                    nc.scalar.activation(s[:], ps[:, col : col + 4], sig)
                    w = wp.tile([HID, 1], f32, tag="w")
                    nc.vector.tensor_scalar(
                        w[:], s[:, 2:3], 0.5, s[:, 0:1], op0=sub, op1=mult
                    )
                    znew = zp.tile([HID, 1], f32, tag="z")
                    nc.vector.tensor_scalar(
                        znew[:], state["z"][:], s[:, 1:2], w[:], op0=mult, op1=add
                    )
                    tcv = tp.tile([HID, 1], f32, tag="tc")
                    nc.scalar.activation(tcv[:], znew[:], tanh, scale=2.0)
                    nc.vector.tensor_scalar(
                        Y[:, t : t + 1], tcv[:], s[:, 3:4], None, op0=mult
                    )
                    state["z"] = znew
                    state["h"] = Y[:, t : t + 1]

        def emit_heads(n):
            zps = hps.tile([NHEAD, chunk], f32, tag="head")
            nc.tensor.matmul(
                zps[:], lhsT=wct_sb[:], rhs=Y[:, ts(n, chunk)], start=True, stop=True
            )
            zst = hsb.tile([NHEAD, chunk], f32, tag="zst")
            nc.vector.tensor_copy(zst[:], zps[:])
            nc.sync.dma_start(Z[:, ts(n, chunk)], zst[:])

        # software-pipelined emission: phase A chunk n+1 overlaps phase B chunk n
        emit_phase_a(0)
        for n in range(nch):
            if n + 1 < nch:
                emit_phase_a(n + 1)
            emit_phase_b(n)
            emit_heads(n)

    nc.compile()
    return nc


def _split_bf16(a):
    import ml_dtypes

    hi = a.astype(ml_dtypes.bfloat16)
    lo = (a - hi.astype(np.float32)).astype(ml_dtypes.bfloat16)
    return np.ascontiguousarray(hi), np.ascontiguousarray(lo)


def _prep_inputs(inputs, mode="bf16"):
    """Host-side folding of masks/biases into weights. Returns per-model maps."""
    import ml_dtypes

    bf = ml_dtypes.bfloat16
    X = np.asarray(inputs["X"], np.float32)[:, 0, :]  # [T, F]
    T = X.shape[0]
    XT = np.ascontiguousarray(X.T)  # [F, T]
    IDT = np.eye(HID, dtype=np.float32)

    W_ih = np.asarray(inputs["W_ih"], np.float32)
    W_hh = np.asarray(inputs["W_hh"], np.float32)
    b_ih = np.asarray(inputs["b_ih"], np.float32)
    b_hh = np.asarray(inputs["b_hh"], np.float32)
    mask_x = np.asarray(inputs["mask_x"], np.float32)
    mask_h = np.asarray(inputs["mask_h"], np.float32)
    heads_w = [np.asarray(inputs[k], np.float32) for k in ("W1", "W2", "W3", "W4")]
    heads_b = [np.asarray(inputs[k], np.float32) for k in ("b1", "b2", "b3", "b4")]

    if mode == "bf16":
        XT_HI, XT_LO = _split_bf16(XT)

    per_model = []
    for r in range(NM):
        wih = (W_ih[r] * mask_x[r][None, :]).copy()
        whh = (W_hh[r] * mask_h[r][None, :]).copy()
        bt = (b_ih[r] + b_hh[r]).copy()
        wih[2 * HID : 3 * HID] *= 2.0
        whh[2 * HID : 3 * HID] *= 2.0
        bt[2 * HID : 3 * HID] *= 2.0
        wc = np.concatenate([w[r] for w in heads_w], axis=0) / NM  # [15, 128]
        if mode == "bf16":
            wih_hi, wih_lo = _split_bf16(np.ascontiguousarray(wih.T))
            bb_hi, bb_lo = _split_bf16(bt[None, :])
            per_model.append(
                {
                    "XT_HI": XT_HI,
                    "XT_LO": XT_LO,
                    "WIH_HI": wih_hi,
                    "WIH_LO": wih_lo,
                    "BB_HI": bb_hi,
                    "BB_LO": bb_lo,
                    "WHH": np.ascontiguousarray(whh.T).astype(bf),
                    "WCT": np.ascontiguousarray(wc.T).astype(bf),
                    "IDT": IDT.astype(bf),
                }
            )
        else:
            per_model.append(
                {
                    "XT": XT,
                    "WIH": np.ascontiguousarray(wih.T),
                    "BB": np.ascontiguousarray(bt[None, :]),
                    "WHH": np.ascontiguousarray(whh.T),
                    "WCT": np.ascontiguousarray(wc.T),
                    "IDT": IDT,
                }
            )
    bias_mean = np.concatenate([b.mean(axis=0) for b in heads_b])  # [15]
    return per_model, bias_mean, T


_CACHE = {}


def _run(inputs, T, mode="bf16", trace=False, n_cores=8):
    from concourse.bass_utils import run_bass_kernel_spmd

    per_model, bias_mean, T_in = _prep_inputs(inputs, mode)
    assert T_in == T
    key = (T, mode)
    if key not in _CACHE:
        _CACHE[key] = _build(T, mode)
    nc = _CACHE[key]
    in_maps = [per_model[min(r, NM - 1)] for r in range(n_cores)]
    res = run_bass_kernel_spmd(nc, in_maps, core_ids=list(range(n_cores)), trace=trace)
    Zsum = np.zeros((NHEAD, T), np.float32)
    for r in range(NM):
        Zsum += res.results[r]["Z"]
    out = Zsum + bias_mean[:, None]
    step_logits = np.ascontiguousarray(out[0:11].T)
    experience = np.ascontiguousarray(out[11:13].T)
    rsd = np.ascontiguousarray(out[13:14].T)
    s = np.ascontiguousarray(out[14:15].T)
    return (step_logits, experience, rsd, s), res


def kernel(**inputs):
    outs, _ = _run(inputs, T_LEN, mode="bf16")
    return outs


# revision 10
# speedup vs baseline: 2.1778x; 1.1470x over previous
"""Trainium2 Bass kernel for the CatRSDNet 5-model MC-dropout LSTM ensemble.

Problem: X [8192,1,1664] -> 5 independent LSTM(1664->128) replicas with
variational dropout masks folded into the weights, 8192 serial timesteps,
then 4 small linear heads ensemble-averaged over the 5 replicas.

Strategy (8 NeuronCores, SPMD, one NEFF):
  - model-parallel: core r simulates ensemble member r (cores 5-7 duplicate
    model 4; their outputs are ignored).
  - Phase A (per core): gx[t] = W_ih_m @ x[t] + b  for all t as large
    matmuls (X transposed on host so F sits on partitions), streamed to a
    DRAM scratch buffer.  fp32 inputs are split into bf16 hi+lo pairs and
    multiplied 3-term (HiHi + HiLo + LoHi) so the PE runs at bf16 rate with
    ~fp24 precision; gx is stored to DRAM as a bf16 hi/lo pair.
  - Phase B: the serial recurrence.  Per step: 4 accumulating 128x128 bf16
    matvecs (W_hh.T stationary, h moving) on top of gx (injected into PSUM
    via identity matmuls, amortized over groups of 4 steps), one Sigmoid
    over all 4 gates (tanh(g) computed as 2*sigmoid(2g)-1 with the g-block
    weights pre-doubled on the host), two fused tensor_scalar ops for the
    cell update (carrying z = c/2 in fp32), one Tanh, one multiply for h.
  - Heads: one [128,15] matmul per 512-step chunk over the stored Y.
Host folds dropout masks + biases into weights, sums per-core head outputs.
"""

import numpy as np

T_LEN = 8192
FEAT = 1664
HID = 128
NM = 5
G4 = 4 * HID  # 512 gates
NHEAD = 15  # 11 + 2 + 1 + 1
KC = FEAT // 128  # 13 contraction tiles for phase A
GROUP = 4  # steps sharing one PSUM gate tile / one gx-inject matmul


def _build(T=T_LEN, mode="bf16"):
    """Build the Bass program. mode: 'bf16' (fast) or 'fp32' (fallback)."""
    from contextlib import ExitStack

    import concourse.bass as bass
    import concourse.tile as tile
    from concourse import bacc, mybir
    from concourse.bass import ts

    f32 = mybir.dt.float32
    bf16 = mybir.dt.bfloat16
    wdt = bf16 if mode == "bf16" else f32
    chunk = min(512, T)
    nch = T // chunk
    assert T % chunk == 0 and chunk % GROUP == 0

    nc = bacc.Bacc("TRN2", target_bir_lowering=False, debug=False, num_devices=8)
    if mode == "bf16":
        XT_HI = nc.dram_tensor("XT_HI", [FEAT, T], bf16, kind="ExternalInput")
        XT_LO = nc.dram_tensor("XT_LO", [FEAT, T], bf16, kind="ExternalInput")
        WIH_HI = nc.dram_tensor("WIH_HI", [FEAT, G4], bf16, kind="ExternalInput")
        WIH_LO = nc.dram_tensor("WIH_LO", [FEAT, G4], bf16, kind="ExternalInput")
        BB_HI = nc.dram_tensor("BB_HI", [1, G4], bf16, kind="ExternalInput")
        BB_LO = nc.dram_tensor("BB_LO", [1, G4], bf16, kind="ExternalInput")
    else:
        XT = nc.dram_tensor("XT", [FEAT, T], f32, kind="ExternalInput")
        WIH = nc.dram_tensor("WIH", [FEAT, G4], f32, kind="ExternalInput")
        BB = nc.dram_tensor("BB", [1, G4], f32, kind="ExternalInput")
    WHH = nc.dram_tensor("WHH", [HID, G4], wdt, kind="ExternalInput")
    WCT = nc.dram_tensor("WCT", [HID, NHEAD], wdt, kind="ExternalInput")
    IDT = nc.dram_tensor("IDT", [HID, HID], wdt, kind="ExternalInput")
    Z = nc.dram_tensor("Z", [NHEAD, T], f32, kind="ExternalOutput")

    sig = mybir.ActivationFunctionType.Sigmoid
    tanh = mybir.ActivationFunctionType.Tanh
    mult = mybir.AluOpType.mult
    add = mybir.AluOpType.add
    sub = mybir.AluOpType.subtract

    with tile.TileContext(nc) as tc, ExitStack() as ctx:
        const = ctx.enter_context(tc.tile_pool(name="const", bufs=1))
        dram = ctx.enter_context(tc.tile_pool(name="dram", bufs=1, space="DRAM"))
        xtp = ctx.enter_context(tc.tile_pool(name="xtp", bufs=2))
        gxps = ctx.enter_context(tc.tile_pool(name="gxps", bufs=2, space="PSUM"))
        gxsb = ctx.enter_context(tc.tile_pool(name="gxsb", bufs=2))
        hsb = ctx.enter_context(tc.tile_pool(name="hsb", bufs=2))
        gxcp = ctx.enter_context(tc.tile_pool(name="gxcp", bufs=2))
        gps = ctx.enter_context(tc.tile_pool(name="gps", bufs=3, space="PSUM"))
        hps = ctx.enter_context(tc.tile_pool(name="hps", bufs=2, space="PSUM"))
        ypool = ctx.enter_context(tc.tile_pool(name="ypool", bufs=1))
        sp = ctx.enter_context(tc.tile_pool(name="sp", bufs=4))
        wp = ctx.enter_context(tc.tile_pool(name="wp", bufs=4))
        zp = ctx.enter_context(tc.tile_pool(name="zp", bufs=4))
        tp = ctx.enter_context(tc.tile_pool(name="tp", bufs=4))

        # ---- constants ----
        whh_sb = const.tile([HID, G4], wdt)
        nc.sync.dma_start(whh_sb[:], WHH[:])
        idt_sb = const.tile([HID, HID], wdt)
        nc.sync.dma_start(idt_sb[:], IDT[:])
        wct_sb = const.tile([HID, NHEAD], wdt)
        nc.sync.dma_start(wct_sb[:], WCT[:])
        if mode == "bf16":
            bbh_sb = const.tile([1, G4], bf16)
            nc.sync.dma_start(bbh_sb[:], BB_HI[:])
            bbl_sb = const.tile([1, G4], bf16)
            nc.sync.dma_start(bbl_sb[:], BB_LO[:])
            wihh_sb = const.tile([128, KC, G4], bf16)
            nc.sync.dma_start(wihh_sb[:], WIH_HI.rearrange("(k p) g -> p k g", p=128)[:])
            wihl_sb = const.tile([128, KC, G4], bf16)
            nc.sync.dma_start(wihl_sb[:], WIH_LO.rearrange("(k p) g -> p k g", p=128)[:])
        else:
            bb_sb = const.tile([1, G4], f32)
            nc.sync.dma_start(bb_sb[:], BB[:])
            wih_sb = const.tile([128, KC, G4], f32)
            nc.sync.dma_start(wih_sb[:], WIH.rearrange("(k p) g -> p k g", p=128)[:])
        ones_sb = const.tile([1, chunk], wdt)
        nc.vector.memset(ones_sb[:], 1.0)
        zero_f32 = const.tile([HID, 1], f32)
        nc.vector.memset(zero_f32[:], 0.0)
        if wdt != f32:
            zero_w = const.tile([HID, 1], wdt)
            nc.vector.memset(zero_w[:], 0.0)
        else:
            zero_w = zero_f32

        if mode == "bf16":
            GXH = dram.tile([4, HID, T], bf16)
            GXL = dram.tile([4, HID, T], bf16)
            gxh_rd = GXH.rearrange("j p t -> p j t")
            gxl_rd = GXL.rearrange("j p t -> p j t")
        else:
            GX = dram.tile([4, HID, T], f32)
            gx_rd = GX.rearrange("j p t -> p j t")

        def emit_phase_a(n):
            if mode == "bf16":
                xth = xtp.tile([128, KC, chunk], bf16, tag="xth")
                nc.sync.dma_start(
                    xth[:],
                    XT_HI.rearrange("(k p) t -> p k t", p=128)[:, :, ts(n, chunk)],
                )
                xtl = xtp.tile([128, KC, chunk], bf16, tag="xtl")
                nc.sync.dma_start(
                    xtl[:],
                    XT_LO.rearrange("(k p) t -> p k t", p=128)[:, :, ts(n, chunk)],
                )
                for j in range(4):
                    ps = gxps.tile([128, chunk], f32, tag="gx")
                    for k in range(KC):
                        for wt, xt_ in (
                            (wihh_sb, xth),
                            (wihh_sb, xtl),
                            (wihl_sb, xth),
                        ):
                            nc.tensor.matmul(
                                ps[:],
                                lhsT=wt[:, k, ts(j, 128)],
                                rhs=xt_[:, k, :],
                                start=(k == 0 and wt is wihh_sb and xt_ is xth),
                                stop=False,
                                skip_group_check=True,
                            )
                    nc.tensor.matmul(
                        ps[:],
                        lhsT=bbh_sb[:, ts(j, 128)],
                        rhs=ones_sb[:],
                        start=False,
                        stop=False,
                        skip_group_check=True,
                    )
                    nc.tensor.matmul(
                        ps[:],
                        lhsT=bbl_sb[:, ts(j, 128)],
                        rhs=ones_sb[:],
                        start=False,
                        stop=True,
                        skip_group_check=True,
                    )
                    sth = gxsb.tile([128, chunk], bf16, tag="sth")
                    nc.vector.tensor_copy(sth[:], ps[:])
                    stw = gxsb.tile([128, chunk], f32, tag="stw")
                    nc.vector.tensor_copy(stw[:], sth[:])
                    stl = gxsb.tile([128, chunk], bf16, tag="stl")
                    nc.vector.tensor_tensor(
                        stl[:], ps[:], stw[:], op=sub
                    )
                    nc.sync.dma_start(GXH[j, :, ts(n, chunk)], sth[:])
                    nc.sync.dma_start(GXL[j, :, ts(n, chunk)], stl[:])
            else:
                xt = xtp.tile([128, KC, chunk], f32, tag="xt")
                nc.sync.dma_start(
                    xt[:], XT.rearrange("(k p) t -> p k t", p=128)[:, :, ts(n, chunk)]
                )
                for j in range(4):
                    ps = gxps.tile([128, chunk], f32, tag="gx")
                    for k in range(KC):
                        nc.tensor.matmul(
                            ps[:],
                            lhsT=wih_sb[:, k, ts(j, 128)],
                            rhs=xt[:, k, :],
                            start=(k == 0),
                            stop=False,
                        )
                    nc.tensor.matmul(
                        ps[:],
                        lhsT=bb_sb[:, ts(j, 128)],
                        rhs=ones_sb[:],
                        start=False,
                        stop=True,
                    )
                    st = gxsb.tile([128, chunk], f32, tag="gxst")
                    nc.vector.tensor_copy(st[:], ps[:])
                    nc.sync.dma_start(GX[j, :, ts(n, chunk)], st[:])

        Y = ypool.tile([HID, T], wdt)
        state = {"c": zero_f32, "h": zero_w}

        def emit_phase_b(n):
            if mode == "bf16":
                gxch = gxcp.tile([128, 4, chunk], bf16, tag="gxch")
                nc.sync.dma_start(gxch[:], gxh_rd[:, :, ts(n, chunk)])
                gxcl = gxcp.tile([128, 4, chunk], bf16, tag="gxcl")
                nc.sync.dma_start(gxcl[:], gxl_rd[:, :, ts(n, chunk)])
                injections = ((gxch, True), (gxcl, False))
            else:
                gxc = gxcp.tile([128, 4, chunk], f32, tag="gxc")
                nc.sync.dma_start(gxc[:], gx_rd[:, :, ts(n, chunk)])
                injections = ((gxc, True),)
            for g0 in range(0, chunk, GROUP):
                ps = gps.tile([128, 4 * GROUP], f32, tag="gates")
                # inject gx for GROUP steps: psum cols laid t-major (t, j)
                for src, is_first in injections:
                    nc.tensor.matmul(
                        ps.rearrange("p (t j) -> p j t", j=4)[:],
                        lhsT=idt_sb[:],
                        rhs=src[:, :, g0 : g0 + GROUP],
                        start=is_first,
                        stop=False,
                        skip_group_check=True,
                    )
                for gl in range(GROUP):
                    t = n * chunk + g0 + gl
                    col = 4 * gl
                    h_prev = state["h"]
                    for j in range(4):
                        nc.tensor.matmul(
                            ps[:, col + j : col + j + 1],
                            lhsT=whh_sb[:, ts(j, 128)],
                            rhs=h_prev[:],
                            start=False,
                            stop=(gl == GROUP - 1 and j == 3),
                            skip_group_check=True,
                        )
                    # s = sigmoid([pre_i, pre_f, 2*pre_g, pre_o])
                    s = sp.tile([HID, 4], f32, tag="s")
                    nc.scalar.activation(s[:], ps[:, col : col + 4], sig)
                    # B = (s_g - 0.5)*s_i = i*g/2   (DVE, overlaps A below)
                    w = wp.tile([HID, 1], f32, tag="w")
                    nc.vector.tensor_scalar(
                        w[:], s[:, 2:3], 0.5, s[:, 0:1], op0=sub, op1=mult
                    )
                    # A = s_f * c_old   (ACT Copy with AP scale, right after s)
                    a = zp.tile([HID, 1], f32, tag="a")
                    nc.scalar.mul(a[:], state["c"][:], s[:, 1:2])
                    # tanh(c_new) = tanh(2B + A)
                    tcv = tp.tile([HID, 1], f32, tag="tc")
                    nc.scalar.activation(tcv[:], w[:], tanh, bias=a[:], scale=2.0)
                    # h = tanh(c_new) * s_o  -> Y
                    nc.vector.tensor_tensor(
                        Y[:, t : t + 1], tcv[:], s[:, 3:4], op=mult
                    )
                    # c_new = 2B + A (off critical path, feeds next step's A)
                    cnew = zp.tile([HID, 1], f32, tag="c")
                    nc.vector.tensor_scalar(
                        cnew[:], w[:], 2.0, a[:], op0=mult, op1=add
                    )
                    state["c"] = cnew
                    state["h"] = Y[:, t : t + 1]

        def emit_heads(n):
            zps = hps.tile([NHEAD, chunk], f32, tag="head")
            nc.tensor.matmul(
                zps[:], lhsT=wct_sb[:], rhs=Y[:, ts(n, chunk)], start=True, stop=True
            )
            zst = hsb.tile([NHEAD, chunk], f32, tag="zst")
            nc.vector.tensor_copy(zst[:], zps[:])
            nc.sync.dma_start(Z[:, ts(n, chunk)], zst[:])

        # software-pipelined emission: phase A chunk n+1 overlaps phase B chunk n
        emit_phase_a(0)
        for n in range(nch):
            if n + 1 < nch:
                emit_phase_a(n + 1)
            emit_phase_b(n)
            emit_heads(n)

    nc.compile()
    return nc


def _split_bf16(a):
    import ml_dtypes

    hi = a.astype(ml_dtypes.bfloat16)
    lo = (a - hi.astype(np.float32)).astype(ml_dtypes.bfloat16)
    return np.ascontiguousarray(hi), np.ascontiguousarray(lo)


def _prep_inputs(inputs, mode="bf16"):
    """Host-side folding of masks/biases into weights. Returns per-model maps."""
    import ml_dtypes

    bf = ml_dtypes.bfloat16
    X = np.asarray(inputs["X"], np.float32)[:, 0, :]  # [T, F]
    T = X.shape[0]
    XT = np.ascontiguousarray(X.T)  # [F, T]
    IDT = np.eye(HID, dtype=np.float32)

    W_ih = np.asarray(inputs["W_ih"], np.float32)
    W_hh = np.asarray(inputs["W_hh"], np.float32)
    b_ih = np.asarray(inputs["b_ih"], np.float32)
    b_hh = np.asarray(inputs["b_hh"], np.float32)
    mask_x = np.asarray(inputs["mask_x"], np.float32)
    mask_h = np.asarray(inputs["mask_h"], np.float32)
    heads_w = [np.asarray(inputs[k], np.float32) for k in ("W1", "W2", "W3", "W4")]
    heads_b = [np.asarray(inputs[k], np.float32) for k in ("b1", "b2", "b3", "b4")]

    if mode == "bf16":
        XT_HI, XT_LO = _split_bf16(XT)

    per_model = []
    for r in range(NM):
        wih = (W_ih[r] * mask_x[r][None, :]).copy()
        whh = (W_hh[r] * mask_h[r][None, :]).copy()
        bt = (b_ih[r] + b_hh[r]).copy()
        wih[2 * HID : 3 * HID] *= 2.0
        whh[2 * HID : 3 * HID] *= 2.0
        bt[2 * HID : 3 * HID] *= 2.0
        wc = np.concatenate([w[r] for w in heads_w], axis=0) / NM  # [15, 128]
        if mode == "bf16":
            wih_hi, wih_lo = _split_bf16(np.ascontiguousarray(wih.T))
            bb_hi, bb_lo = _split_bf16(bt[None, :])
            per_model.append(
                {
                    "XT_HI": XT_HI,
                    "XT_LO": XT_LO,
                    "WIH_HI": wih_hi,
                    "WIH_LO": wih_lo,
                    "BB_HI": bb_hi,
                    "BB_LO": bb_lo,
                    "WHH": np.ascontiguousarray(whh.T).astype(bf),
                    "WCT": np.ascontiguousarray(wc.T).astype(bf),
                    "IDT": IDT.astype(bf),
                }
            )
        else:
            per_model.append(
                {
                    "XT": XT,
                    "WIH": np.ascontiguousarray(wih.T),
                    "BB": np.ascontiguousarray(bt[None, :]),
                    "WHH": np.ascontiguousarray(whh.T),
                    "WCT": np.ascontiguousarray(wc.T),
                    "IDT": IDT,
                }
            )
    bias_mean = np.concatenate([b.mean(axis=0) for b in heads_b])  # [15]
    return per_model, bias_mean, T


_CACHE = {}


def _run(inputs, T, mode="bf16", trace=False, n_cores=8):
    from concourse.bass_utils import run_bass_kernel_spmd

    per_model, bias_mean, T_in = _prep_inputs(inputs, mode)
    assert T_in == T
    key = (T, mode)
    if key not in _CACHE:
        _CACHE[key] = _build(T, mode)
    nc = _CACHE[key]
    in_maps = [per_model[min(r, NM - 1)] for r in range(n_cores)]
    res = run_bass_kernel_spmd(nc, in_maps, core_ids=list(range(n_cores)), trace=trace)
    Zsum = np.zeros((NHEAD, T), np.float32)
    for r in range(NM):
        Zsum += res.results[r]["Z"]
    out = Zsum + bias_mean[:, None]
    step_logits = np.ascontiguousarray(out[0:11].T)
    experience = np.ascontiguousarray(out[11:13].T)
    rsd = np.ascontiguousarray(out[13:14].T)
    s = np.ascontiguousarray(out[14:15].T)
    return (step_logits, experience, rsd, s), res


def kernel(**inputs):
    outs, _ = _run(inputs, T_LEN, mode="bf16")
    return outs


# revision 12
# speedup vs baseline: 2.2003x; 1.0103x over previous
"""Trainium2 Bass kernel for the CatRSDNet 5-model MC-dropout LSTM ensemble.

Problem: X [8192,1,1664] -> 5 independent LSTM(1664->128) replicas with
variational dropout masks folded into the weights, 8192 serial timesteps,
then 4 small linear heads ensemble-averaged over the 5 replicas.

Strategy (8 NeuronCores, SPMD, one NEFF):
  - model-parallel: core r simulates ensemble member r (cores 5-7 duplicate
    model 4; their outputs are ignored).
  - Phase A (per core): gx[t] = W_ih_m @ x[t] + b  for all t as large
    matmuls (X transposed on host so F sits on partitions), streamed to a
    DRAM scratch buffer.  fp32 inputs are split into bf16 hi+lo pairs and
    multiplied 3-term (HiHi + HiLo + LoHi) so the PE runs at bf16 rate with
    ~fp24 precision; gx is stored to DRAM as a bf16 hi/lo pair.
  - Phase B: the serial recurrence.  Per step: 4 accumulating 128x128 bf16
    matvecs (W_hh.T stationary, h moving) on top of gx (injected into PSUM
    via identity matmuls, amortized over groups of 4 steps), one Sigmoid
    over all 4 gates (tanh(g) computed as 2*sigmoid(2g)-1 with the g-block
    weights pre-doubled on the host), two fused tensor_scalar ops for the
    cell update (carrying z = c/2 in fp32), one Tanh, one multiply for h.
  - Heads: one [128,15] matmul per 512-step chunk over the stored Y.
Host folds dropout masks + biases into weights, sums per-core head outputs.
"""

import numpy as np

T_LEN = 8192
FEAT = 1664
HID = 128
NM = 5
G4 = 4 * HID  # 512 gates
NHEAD = 15  # 11 + 2 + 1 + 1
KC = FEAT // 128  # 13 contraction tiles for phase A
GROUP = 4  # steps sharing one PSUM gate tile / one gx-inject matmul


def _build(T=T_LEN, mode="bf16"):
    """Build the Bass program. mode: 'bf16' (fast) or 'fp32' (fallback)."""
    from contextlib import ExitStack

    import concourse.bass as bass
    import concourse.tile as tile
    from concourse import bacc, mybir
    from concourse.bass import ts

    f32 = mybir.dt.float32
    bf16 = mybir.dt.bfloat16
    wdt = bf16 if mode == "bf16" else f32
    chunk = min(512, T)
    nch = T // chunk
    assert T % chunk == 0 and chunk % GROUP == 0

    nc = bacc.Bacc("TRN2", target_bir_lowering=False, debug=False, num_devices=8)
    if mode == "bf16":
        XT_HI = nc.dram_tensor("XT_HI", [FEAT, T], bf16, kind="ExternalInput")
        XT_LO = nc.dram_tensor("XT_LO", [FEAT, T], bf16, kind="ExternalInput")
        WIH_HI = nc.dram_tensor("WIH_HI", [FEAT, G4], bf16, kind="ExternalInput")
        WIH_LO = nc.dram_tensor("WIH_LO", [FEAT, G4], bf16, kind="ExternalInput")
        BB_HI = nc.dram_tensor("BB_HI", [1, G4], bf16, kind="ExternalInput")
        BB_LO = nc.dram_tensor("BB_LO", [1, G4], bf16, kind="ExternalInput")
    else:
        XT = nc.dram_tensor("XT", [FEAT, T], f32, kind="ExternalInput")
        WIH = nc.dram_tensor("WIH", [FEAT, G4], f32, kind="ExternalInput")
        BB = nc.dram_tensor("BB", [1, G4], f32, kind="ExternalInput")
    WHH = nc.dram_tensor("WHH", [HID, G4], wdt, kind="ExternalInput")
    WCT = nc.dram_tensor("WCT", [HID, NHEAD], wdt, kind="ExternalInput")
    IDT = nc.dram_tensor("IDT", [HID, HID], wdt, kind="ExternalInput")
    Z = nc.dram_tensor("Z", [NHEAD, T], f32, kind="ExternalOutput")

    sig = mybir.ActivationFunctionType.Sigmoid
    tanh = mybir.ActivationFunctionType.Tanh
    mult = mybir.AluOpType.mult
    add = mybir.AluOpType.add
    sub = mybir.AluOpType.subtract

    with tile.TileContext(nc) as tc, ExitStack() as ctx:
        const = ctx.enter_context(tc.tile_pool(name="const", bufs=1))
        dram = ctx.enter_context(tc.tile_pool(name="dram", bufs=1, space="DRAM"))
        xtp = ctx.enter_context(tc.tile_pool(name="xtp", bufs=2))
        gxps = ctx.enter_context(tc.tile_pool(name="gxps", bufs=2, space="PSUM"))
        gxsb = ctx.enter_context(tc.tile_pool(name="gxsb", bufs=2))
        hsb = ctx.enter_context(tc.tile_pool(name="hsb", bufs=2))
        gxcp = ctx.enter_context(tc.tile_pool(name="gxcp", bufs=2))
        gps = ctx.enter_context(tc.tile_pool(name="gps", bufs=3, space="PSUM"))
        hps = ctx.enter_context(tc.tile_pool(name="hps", bufs=2, space="PSUM"))
        ypool = ctx.enter_context(tc.tile_pool(name="ypool", bufs=1))
        sp = ctx.enter_context(tc.tile_pool(name="sp", bufs=4))
        wp = ctx.enter_context(tc.tile_pool(name="wp", bufs=4))
        zp = ctx.enter_context(tc.tile_pool(name="zp", bufs=4))
        tp = ctx.enter_context(tc.tile_pool(name="tp", bufs=4))

        # ---- constants ----
        whh_sb = const.tile([HID, G4], wdt)
        nc.sync.dma_start(whh_sb[:], WHH[:])
        idt_sb = const.tile([HID, HID], wdt)
        nc.sync.dma_start(idt_sb[:], IDT[:])
        wct_sb = const.tile([HID, NHEAD], wdt)
        nc.sync.dma_start(wct_sb[:], WCT[:])
        if mode == "bf16":
            bbh_sb = const.tile([1, G4], bf16)
            nc.sync.dma_start(bbh_sb[:], BB_HI[:])
            bbl_sb = const.tile([1, G4], bf16)
            nc.sync.dma_start(bbl_sb[:], BB_LO[:])
            wihh_sb = const.tile([128, KC, G4], bf16)
            nc.sync.dma_start(wihh_sb[:], WIH_HI.rearrange("(k p) g -> p k g", p=128)[:])
            wihl_sb = const.tile([128, KC, G4], bf16)
            nc.sync.dma_start(wihl_sb[:], WIH_LO.rearrange("(k p) g -> p k g", p=128)[:])
        else:
            bb_sb = const.tile([1, G4], f32)
            nc.sync.dma_start(bb_sb[:], BB[:])
            wih_sb = const.tile([128, KC, G4], f32)
            nc.sync.dma_start(wih_sb[:], WIH.rearrange("(k p) g -> p k g", p=128)[:])
        ones_sb = const.tile([1, chunk], wdt)
        nc.vector.memset(ones_sb[:], 1.0)
        zero_f32 = const.tile([HID, 1], f32)
        nc.vector.memset(zero_f32[:], 0.0)
        if wdt != f32:
            zero_w = const.tile([HID, 1], wdt)
            nc.vector.memset(zero_w[:], 0.0)
        else:
            zero_w = zero_f32

        if mode == "bf16":
            GXH = dram.tile([4, HID, T], bf16)
            GXL = dram.tile([4, HID, T], bf16)
            gxh_rd = GXH.rearrange("j p t -> p j t")
            gxl_rd = GXL.rearrange("j p t -> p j t")
        else:
            GX = dram.tile([4, HID, T], f32)
            gx_rd = GX.rearrange("j p t -> p j t")

        def emit_phase_a(n):
            if mode == "bf16":
                xth = xtp.tile([128, KC, chunk], bf16, tag="xth")
                nc.sync.dma_start(
                    xth[:],
                    XT_HI.rearrange("(k p) t -> p k t", p=128)[:, :, ts(n, chunk)],
                )
                xtl = xtp.tile([128, KC, chunk], bf16, tag="xtl")
                nc.sync.dma_start(
                    xtl[:],
                    XT_LO.rearrange("(k p) t -> p k t", p=128)[:, :, ts(n, chunk)],
                )
                for j in range(4):
                    ps = gxps.tile([128, chunk], f32, tag="gx")
                    for k in range(KC):
                        for wt, xt_ in (
                            (wihh_sb, xth),
                            (wihh_sb, xtl),
                            (wihl_sb, xth),
                        ):
                            nc.tensor.matmul(
                                ps[:],
                                lhsT=wt[:, k, ts(j, 128)],
                                rhs=xt_[:, k, :],
                                start=(k == 0 and wt is wihh_sb and xt_ is xth),
                                stop=False,
                                skip_group_check=True,
                            )
                    nc.tensor.matmul(
                        ps[:],
                        lhsT=bbh_sb[:, ts(j, 128)],
                        rhs=ones_sb[:],
                        start=False,
                        stop=False,
                        skip_group_check=True,
                    )
                    nc.tensor.matmul(
                        ps[:],
                        lhsT=bbl_sb[:, ts(j, 128)],
                        rhs=ones_sb[:],
                        start=False,
                        stop=True,
                        skip_group_check=True,
                    )
                    sth = gxsb.tile([128, chunk], bf16, tag="sth")
                    nc.vector.tensor_copy(sth[:], ps[:])
                    stw = gxsb.tile([128, chunk], f32, tag="stw")
                    nc.vector.tensor_copy(stw[:], sth[:])
                    stl = gxsb.tile([128, chunk], bf16, tag="stl")
                    nc.vector.tensor_tensor(
                        stl[:], ps[:], stw[:], op=sub
                    )
                    nc.sync.dma_start(GXH[j, :, ts(n, chunk)], sth[:])
                    nc.sync.dma_start(GXL[j, :, ts(n, chunk)], stl[:])
            else:
                xt = xtp.tile([128, KC, chunk], f32, tag="xt")
                nc.sync.dma_start(
                    xt[:], XT.rearrange("(k p) t -> p k t", p=128)[:, :, ts(n, chunk)]
                )
                for j in range(4):
                    ps = gxps.tile([128, chunk], f32, tag="gx")
                    for k in range(KC):
                        nc.tensor.matmul(
                            ps[:],
                            lhsT=wih_sb[:, k, ts(j, 128)],
                            rhs=xt[:, k, :],
                            start=(k == 0),
                            stop=False,
                        )
                    nc.tensor.matmul(
                        ps[:],
                        lhsT=bb_sb[:, ts(j, 128)],
                        rhs=ones_sb[:],
                        start=False,
                        stop=True,
                    )
                    st = gxsb.tile([128, chunk], f32, tag="gxst")
                    nc.vector.tensor_copy(st[:], ps[:])
                    nc.sync.dma_start(GX[j, :, ts(n, chunk)], st[:])

        Y = ypool.tile([HID, T], wdt)
        state = {"c": zero_f32, "h": zero_w}

        # s-tile ring: col0 = constant 2.0, cols 1-4 = sigmoid outputs
        # [i, f, g2, o].  tensor_tensor_scan over cols [0,1] with
        # initial=s_g computes (2*s_g - 1)*s_i = i*g in one DVE op.
        s_ring = []
        for i_ in range(4):
            st_ = sp.tile([HID, 5], f32, name=f"sring{i_}", tag=f"sring{i_}")
            nc.vector.memset(st_[:, 0:1], 2.0)
            s_ring.append(st_)
        scan_c2 = const.tile([HID, 2], f32)
        nc.vector.memset(scan_c2[:, 0:1], -1.0)
        nc.vector.memset(scan_c2[:, 1:2], 0.0)

        def emit_phase_b(n):
            if mode == "bf16":
                gxch = gxcp.tile([128, 4, chunk], bf16, tag="gxch")
                nc.sync.dma_start(gxch[:], gxh_rd[:, :, ts(n, chunk)])
                gxcl = gxcp.tile([128, 4, chunk], bf16, tag="gxcl")
                nc.sync.dma_start(gxcl[:], gxl_rd[:, :, ts(n, chunk)])
                injections = ((gxch, True), (gxcl, False))
            else:
                gxc = gxcp.tile([128, 4, chunk], f32, tag="gxc")
                nc.sync.dma_start(gxc[:], gx_rd[:, :, ts(n, chunk)])
                injections = ((gxc, True),)
            for g0 in range(0, chunk, GROUP):
                ps = gps.tile([128, 4 * GROUP], f32, tag="gates")
                # inject gx for GROUP steps: psum cols laid t-major (t, j)
                for src, is_first in injections:
                    nc.tensor.matmul(
                        ps.rearrange("p (t j) -> p j t", j=4)[:],
                        lhsT=idt_sb[:],
                        rhs=src[:, :, g0 : g0 + GROUP],
                        start=is_first,
                        stop=False,
                        skip_group_check=True,
                    )
                for gl in range(GROUP):
                    t = n * chunk + g0 + gl
                    col = 4 * gl
                    h_prev = state["h"]
                    for j in range(4):
                        nc.tensor.matmul(
                            ps[:, col + j : col + j + 1],
                            lhsT=whh_sb[:, ts(j, 128)],
                            rhs=h_prev[:],
                            start=False,
                            stop=(gl == GROUP - 1 and j == 3),
                            skip_group_check=True,
                        )
                    # s cols 1-4 = sigmoid([pre_i, pre_f, 2*pre_g, pre_o])
                    s = s_ring[(t // 1) % 4]
                    nc.scalar.activation(s[:, 1:5], ps[:, col : col + 4], sig)
                    # ig = (2*s_g - 1)*s_i = i*g via length-2 scan:
                    #   state=s_g; state=2*state-1; state=s_i*state+0
                    b2 = wp.tile([HID, 2], f32, tag="b2")
                    nc.vector.tensor_tensor_scan(
                        b2[:], s[:, 0:2], scan_c2[:], s[:, 3:4], op0=mult, op1=add
                    )
                    # tanh(c_new) = tanh(s_f * c_old + i*g)
                    tcv = tp.tile([HID, 1], f32, tag="tc")
                    nc.scalar.activation(
                        tcv[:], state["c"][:], tanh, bias=b2[:, 1:2], scale=s[:, 2:3]
                    )
                    # h = tanh(c_new) * s_o  -> Y
                    nc.vector.tensor_tensor(
                        Y[:, t : t + 1], tcv[:], s[:, 4:5], op=mult
                    )
                    # c_new = s_f*c_old + i*g (off critical path)
                    cnew = zp.tile([HID, 1], f32, tag="c")
                    nc.vector.tensor_scalar(
                        cnew[:], state["c"][:], s[:, 2:3], b2[:, 1:2], op0=mult, op1=add
                    )
                    state["c"] = cnew
                    state["h"] = Y[:, t : t + 1]

        def emit_heads(n):
            zps = hps.tile([NHEAD, chunk], f32, tag="head")
            nc.tensor.matmul(
                zps[:], lhsT=wct_sb[:], rhs=Y[:, ts(n, chunk)], start=True, stop=True
            )
            zst = hsb.tile([NHEAD, chunk], f32, tag="zst")
            nc.vector.tensor_copy(zst[:], zps[:])
            nc.sync.dma_start(Z[:, ts(n, chunk)], zst[:])

        # software-pipelined emission: phase A chunk n+1 overlaps phase B chunk n
        emit_phase_a(0)
        for n in range(nch):
            if n + 1 < nch:
                emit_phase_a(n + 1)
            emit_phase_b(n)
            emit_heads(n)

    nc.compile()
    return nc


def _split_bf16(a):
    import ml_dtypes

    hi = a.astype(ml_dtypes.bfloat16)
    lo = (a - hi.astype(np.float32)).astype(ml_dtypes.bfloat16)
    return np.ascontiguousarray(hi), np.ascontiguousarray(lo)


def _prep_inputs(inputs, mode="bf16"):
    """Host-side folding of masks/biases into weights. Returns per-model maps."""
    import ml_dtypes

    bf = ml_dtypes.bfloat16
    X = np.asarray(inputs["X"], np.float32)[:, 0, :]  # [T, F]
    T = X.shape[0]
    XT = np.ascontiguousarray(X.T)  # [F, T]
    IDT = np.eye(HID, dtype=np.float32)

    W_ih = np.asarray(inputs["W_ih"], np.float32)
    W_hh = np.asarray(inputs["W_hh"], np.float32)
    b_ih = np.asarray(inputs["b_ih"], np.float32)
    b_hh = np.asarray(inputs["b_hh"], np.float32)
    mask_x = np.asarray(inputs["mask_x"], np.float32)
    mask_h = np.asarray(inputs["mask_h"], np.float32)
    heads_w = [np.asarray(inputs[k], np.float32) for k in ("W1", "W2", "W3", "W4")]
    heads_b = [np.asarray(inputs[k], np.float32) for k in ("b1", "b2", "b3", "b4")]

    if mode == "bf16":
        XT_HI, XT_LO = _split_bf16(XT)

    per_model = []
    for r in range(NM):
        wih = (W_ih[r] * mask_x[r][None, :]).copy()
        whh = (W_hh[r] * mask_h[r][None, :]).copy()
        bt = (b_ih[r] + b_hh[r]).copy()
        wih[2 * HID : 3 * HID] *= 2.0
        whh[2 * HID : 3 * HID] *= 2.0
        bt[2 * HID : 3 * HID] *= 2.0
        wc = np.concatenate([w[r] for w in heads_w], axis=0) / NM  # [15, 128]
        if mode == "bf16":
            wih_hi, wih_lo = _split_bf16(np.ascontiguousarray(wih.T))
            bb_hi, bb_lo = _split_bf16(bt[None, :])
            per_model.append(
                {
                    "XT_HI": XT_HI,
                    "XT_LO": XT_LO,
                    "WIH_HI": wih_hi,
                    "WIH_LO": wih_lo,
                    "BB_HI": bb_hi,
                    "BB_LO": bb_lo,
                    "WHH": np.ascontiguousarray(whh.T).astype(bf),
                    "WCT": np.ascontiguousarray(wc.T).astype(bf),
                    "IDT": IDT.astype(bf),
                }
            )
        else:
            per_model.append(
                {
                    "XT": XT,
                    "WIH": np.ascontiguousarray(wih.T),
                    "BB": np.ascontiguousarray(bt[None, :]),
                    "WHH": np.ascontiguousarray(whh.T),
                    "WCT": np.ascontiguousarray(wc.T),
                    "IDT": IDT,
                }
            )
    bias_mean = np.concatenate([b.mean(axis=0) for b in heads_b])  # [15]
    return per_model, bias_mean, T


_CACHE = {}


def _run(inputs, T, mode="bf16", trace=False, n_cores=8):
    from concourse.bass_utils import run_bass_kernel_spmd

    per_model, bias_mean, T_in = _prep_inputs(inputs, mode)
    assert T_in == T
    key = (T, mode)
    if key not in _CACHE:
        _CACHE[key] = _build(T, mode)
    nc = _CACHE[key]
    in_maps = [per_model[min(r, NM - 1)] for r in range(n_cores)]
    res = run_bass_kernel_spmd(nc, in_maps, core_ids=list(range(n_cores)), trace=trace)
    Zsum = np.zeros((NHEAD, T), np.float32)
    for r in range(NM):
        Zsum += res.results[r]["Z"]
    out = Zsum + bias_mean[:, None]
    step_logits = np.ascontiguousarray(out[0:11].T)
    experience = np.ascontiguousarray(out[11:13].T)
    rsd = np.ascontiguousarray(out[13:14].T)
    s = np.ascontiguousarray(out[14:15].T)
    return (step_logits, experience, rsd, s), res


def kernel(**inputs):
    outs, _ = _run(inputs, T_LEN, mode="bf16")
    return outs


# revision 15
# speedup vs baseline: 2.2014x; 1.0005x over previous
"""Trainium2 Bass kernel for the CatRSDNet 5-model MC-dropout LSTM ensemble.

Problem: X [8192,1,1664] -> 5 independent LSTM(1664->128) replicas with
variational dropout masks folded into the weights, 8192 serial timesteps,
then 4 small linear heads ensemble-averaged over the 5 replicas.

Strategy (8 NeuronCores, SPMD, one NEFF):
  - model-parallel: core r simulates ensemble member r (cores 5-7 duplicate
    model 4; their outputs are ignored).
  - Phase A (per core): gx[t] = W_ih_m @ x[t] + b  for all t as large
    matmuls (X transposed on host so F sits on partitions), streamed to a
    DRAM scratch buffer.  fp32 inputs are split into bf16 hi+lo pairs and
    multiplied 3-term (HiHi + HiLo + LoHi) so the PE runs at bf16 rate with
    ~fp24 precision; gx is stored to DRAM as a bf16 hi/lo pair.
  - Phase B: the serial recurrence.  Per step: 4 accumulating 128x128 bf16
    matvecs (W_hh.T stationary, h moving) on top of gx (injected into PSUM
    via identity matmuls, amortized over groups of 4 steps), one Sigmoid
    over all 4 gates (tanh(g) computed as 2*sigmoid(2g)-1 with the g-block
    weights pre-doubled on the host), i*g = (2*s_g-1)*s_i in a single
    tensor_tensor_scan (constant columns baked next to the sigmoid outputs),
    tanh(c_new) in one Tanh with per-partition AP scale (s_f) and bias (i*g)
    applied to c_old, one tensor_tensor for h; the fp32 cell state c_new is
    materialized off the critical path.
  - Heads: one [128,15] matmul per 512-step chunk over the stored Y.
Host folds dropout masks + biases into weights, sums per-core head outputs.
"""

import numpy as np

T_LEN = 8192
FEAT = 1664
HID = 128
NM = 5
G4 = 4 * HID  # 512 gates
NHEAD = 15  # 11 + 2 + 1 + 1
KC = FEAT // 128  # 13 contraction tiles for phase A
GROUP = 4  # steps sharing one PSUM gate tile / one gx-inject matmul


def _build(T=T_LEN, mode="bf16"):
    """Build the Bass program. mode: 'bf16' (fast) or 'fp32' (fallback)."""
    from contextlib import ExitStack

    import concourse.bass as bass
    import concourse.tile as tile
    from concourse import bacc, mybir
    from concourse.bass import ts

    f32 = mybir.dt.float32
    bf16 = mybir.dt.bfloat16
    wdt = bf16 if mode == "bf16" else f32
    chunk = min(512, T)
    nch = T // chunk
    assert T % chunk == 0 and chunk % GROUP == 0

    nc = bacc.Bacc("TRN2", target_bir_lowering=False, debug=False, num_devices=8)
    if mode == "bf16":
        XT_HI = nc.dram_tensor("XT_HI", [FEAT, T], bf16, kind="ExternalInput")
        XT_LO = nc.dram_tensor("XT_LO", [FEAT, T], bf16, kind="ExternalInput")
        WIH_HI = nc.dram_tensor("WIH_HI", [FEAT, G4], bf16, kind="ExternalInput")
        WIH_LO = nc.dram_tensor("WIH_LO", [FEAT, G4], bf16, kind="ExternalInput")
        BB_HI = nc.dram_tensor("BB_HI", [1, G4], bf16, kind="ExternalInput")
        BB_LO = nc.dram_tensor("BB_LO", [1, G4], bf16, kind="ExternalInput")
    else:
        XT = nc.dram_tensor("XT", [FEAT, T], f32, kind="ExternalInput")
        WIH = nc.dram_tensor("WIH", [FEAT, G4], f32, kind="ExternalInput")
        BB = nc.dram_tensor("BB", [1, G4], f32, kind="ExternalInput")
    WHH = nc.dram_tensor("WHH", [HID, G4], wdt, kind="ExternalInput")
    WCT = nc.dram_tensor("WCT", [HID, NHEAD], wdt, kind="ExternalInput")
    IDT = nc.dram_tensor("IDT", [HID, HID], wdt, kind="ExternalInput")
    Z = nc.dram_tensor("Z", [NHEAD, T], f32, kind="ExternalOutput")

    sig = mybir.ActivationFunctionType.Sigmoid
    tanh = mybir.ActivationFunctionType.Tanh
    mult = mybir.AluOpType.mult
    add = mybir.AluOpType.add
    sub = mybir.AluOpType.subtract

    with tile.TileContext(nc) as tc, ExitStack() as ctx:
        const = ctx.enter_context(tc.tile_pool(name="const", bufs=1))
        dram = ctx.enter_context(tc.tile_pool(name="dram", bufs=1, space="DRAM"))
        xtp = ctx.enter_context(tc.tile_pool(name="xtp", bufs=2))
        gxps = ctx.enter_context(tc.tile_pool(name="gxps", bufs=2, space="PSUM"))
        gxsb = ctx.enter_context(tc.tile_pool(name="gxsb", bufs=2))
        hsb = ctx.enter_context(tc.tile_pool(name="hsb", bufs=2))
        gxcp = ctx.enter_context(tc.tile_pool(name="gxcp", bufs=2))
        gps = ctx.enter_context(tc.tile_pool(name="gps", bufs=3, space="PSUM"))
        hps = ctx.enter_context(tc.tile_pool(name="hps", bufs=2, space="PSUM"))
        ypool = ctx.enter_context(tc.tile_pool(name="ypool", bufs=1))
        sp = ctx.enter_context(tc.tile_pool(name="sp", bufs=4))
        wp = ctx.enter_context(tc.tile_pool(name="wp", bufs=4))
        zp = ctx.enter_context(tc.tile_pool(name="zp", bufs=4))
        tp = ctx.enter_context(tc.tile_pool(name="tp", bufs=4))

        # ---- constants ----
        whh_sb = const.tile([HID, G4], wdt)
        nc.sync.dma_start(whh_sb[:], WHH[:])
        idt_sb = const.tile([HID, HID], wdt)
        nc.sync.dma_start(idt_sb[:], IDT[:])
        wct_sb = const.tile([HID, NHEAD], wdt)
        nc.sync.dma_start(wct_sb[:], WCT[:])
        if mode == "bf16":
            bbh_sb = const.tile([1, G4], bf16)
            nc.sync.dma_start(bbh_sb[:], BB_HI[:])
            bbl_sb = const.tile([1, G4], bf16)
            nc.sync.dma_start(bbl_sb[:], BB_LO[:])
            wihh_sb = const.tile([128, KC, G4], bf16)
            nc.sync.dma_start(wihh_sb[:], WIH_HI.rearrange("(k p) g -> p k g", p=128)[:])
            wihl_sb = const.tile([128, KC, G4], bf16)
            nc.sync.dma_start(wihl_sb[:], WIH_LO.rearrange("(k p) g -> p k g", p=128)[:])
        else:
            bb_sb = const.tile([1, G4], f32)
            nc.sync.dma_start(bb_sb[:], BB[:])
            wih_sb = const.tile([128, KC, G4], f32)
            nc.sync.dma_start(wih_sb[:], WIH.rearrange("(k p) g -> p k g", p=128)[:])
        ones_sb = const.tile([1, chunk], wdt)
        nc.vector.memset(ones_sb[:], 1.0)
        zero_f32 = const.tile([HID, 1], f32)
        nc.vector.memset(zero_f32[:], 0.0)
        if wdt != f32:
            zero_w = const.tile([HID, 1], wdt)
            nc.vector.memset(zero_w[:], 0.0)
        else:
            zero_w = zero_f32

        if mode == "bf16":
            GXH = dram.tile([4, HID, T], bf16)
            GXL = dram.tile([4, HID, T], bf16)
            gxh_rd = GXH.rearrange("j p t -> p j t")
            gxl_rd = GXL.rearrange("j p t -> p j t")
        else:
            GX = dram.tile([4, HID, T], f32)
            gx_rd = GX.rearrange("j p t -> p j t")

        def emit_phase_a(n):
            if mode == "bf16":
                xth = xtp.tile([128, KC, chunk], bf16, tag="xth")
                nc.sync.dma_start(
                    xth[:],
                    XT_HI.rearrange("(k p) t -> p k t", p=128)[:, :, ts(n, chunk)],
                )
                xtl = xtp.tile([128, KC, chunk], bf16, tag="xtl")
                nc.sync.dma_start(
                    xtl[:],
                    XT_LO.rearrange("(k p) t -> p k t", p=128)[:, :, ts(n, chunk)],
                )
                for j in range(4):
                    ps = gxps.tile([128, chunk], f32, tag="gx")
                    for k in range(KC):
                        for wt, xt_ in (
                            (wihh_sb, xth),
                            (wihh_sb, xtl),
                            (wihl_sb, xth),
                        ):
                            nc.tensor.matmul(
                                ps[:],
                                lhsT=wt[:, k, ts(j, 128)],
                                rhs=xt_[:, k, :],
                                start=(k == 0 and wt is wihh_sb and xt_ is xth),
                                stop=False,
                                skip_group_check=True,
                            )
                    nc.tensor.matmul(
                        ps[:],
                        lhsT=bbh_sb[:, ts(j, 128)],
                        rhs=ones_sb[:],
                        start=False,
                        stop=False,
                        skip_group_check=True,
                    )
                    nc.tensor.matmul(
                        ps[:],
                        lhsT=bbl_sb[:, ts(j, 128)],
                        rhs=ones_sb[:],
                        start=False,
                        stop=True,
                        skip_group_check=True,
                    )
                    sth = gxsb.tile([128, chunk], bf16, tag="sth")
                    nc.vector.tensor_copy(sth[:], ps[:])
                    stw = gxsb.tile([128, chunk], f32, tag="stw")
                    nc.vector.tensor_copy(stw[:], sth[:])
                    stl = gxsb.tile([128, chunk], bf16, tag="stl")
                    nc.vector.tensor_tensor(
                        stl[:], ps[:], stw[:], op=sub
                    )
                    nc.sync.dma_start(GXH[j, :, ts(n, chunk)], sth[:])
                    nc.sync.dma_start(GXL[j, :, ts(n, chunk)], stl[:])
            else:
                xt = xtp.tile([128, KC, chunk], f32, tag="xt")
                nc.sync.dma_start(
                    xt[:], XT.rearrange("(k p) t -> p k t", p=128)[:, :, ts(n, chunk)]
                )
                for j in range(4):
                    ps = gxps.tile([128, chunk], f32, tag="gx")
                    for k in range(KC):
                        nc.tensor.matmul(
                            ps[:],
                            lhsT=wih_sb[:, k, ts(j, 128)],
                            rhs=xt[:, k, :],
                            start=(k == 0),
                            stop=False,
                        )
                    nc.tensor.matmul(
                        ps[:],
                        lhsT=bb_sb[:, ts(j, 128)],
                        rhs=ones_sb[:],
                        start=False,
                        stop=True,
                    )
                    st = gxsb.tile([128, chunk], f32, tag="gxst")
                    nc.vector.tensor_copy(st[:], ps[:])
                    nc.sync.dma_start(GX[j, :, ts(n, chunk)], st[:])

        Y = ypool.tile([HID, T], wdt)
        state = {"c": zero_f32, "h": zero_w}

        # s-tile ring: col0 = constant 2.0, cols 1-4 = sigmoid outputs
        # [i, f, g2, o].  tensor_tensor_scan over cols [0,1] with
        # initial=s_g computes (2*s_g - 1)*s_i = i*g in one DVE op.
        s_ring = []
        for i_ in range(4):
            st_ = sp.tile([HID, 5], f32, name=f"sring{i_}", tag=f"sring{i_}")
            nc.vector.memset(st_[:, 0:1], 2.0)
            s_ring.append(st_)
        scan_c2 = const.tile([HID, 2], f32)
        nc.vector.memset(scan_c2[:, 0:1], -1.0)
        nc.vector.memset(scan_c2[:, 1:2], 0.0)

        def emit_phase_b(n):
            if mode == "bf16":
                gxch = gxcp.tile([128, 4, chunk], bf16, tag="gxch")
                nc.sync.dma_start(gxch[:], gxh_rd[:, :, ts(n, chunk)])
                gxcl = gxcp.tile([128, 4, chunk], bf16, tag="gxcl")
                nc.sync.dma_start(gxcl[:], gxl_rd[:, :, ts(n, chunk)])
                injections = ((gxch, True), (gxcl, False))
            else:
                gxc = gxcp.tile([128, 4, chunk], f32, tag="gxc")
                nc.sync.dma_start(gxc[:], gx_rd[:, :, ts(n, chunk)])
                injections = ((gxc, True),)
            for g0 in range(0, chunk, GROUP):
                ps = gps.tile([128, 4 * GROUP], f32, tag="gates")
                # inject gx for GROUP steps: psum cols laid t-major (t, j)
                for src, is_first in injections:
                    nc.tensor.matmul(
                        ps.rearrange("p (t j) -> p j t", j=4)[:],
                        lhsT=idt_sb[:],
                        rhs=src[:, :, g0 : g0 + GROUP],
                        start=is_first,
                        stop=False,
                        skip_group_check=True,
                    )
                for gl in range(GROUP):
                    t = n * chunk + g0 + gl
                    col = 4 * gl
                    h_prev = state["h"]
                    for j in range(4):
                        nc.tensor.matmul(
                            ps[:, col + j : col + j + 1],
                            lhsT=whh_sb[:, ts(j, 128)],
                            rhs=h_prev[:],
                            start=False,
                            stop=(gl == GROUP - 1 and j == 3),
                            skip_group_check=True,
                        )
                    # s cols 1-4 = sigmoid([pre_i, pre_f, 2*pre_g, pre_o])
                    s = s_ring[(t // 1) % 4]
                    nc.scalar.activation(s[:, 1:5], ps[:, col : col + 4], sig)
                    # ig = (2*s_g - 1)*s_i = i*g via length-2 scan:
                    #   state=s_g; state=2*state-1; state=s_i*state+0
                    b2 = wp.tile([HID, 2], f32, tag="b2")
                    nc.vector.tensor_tensor_scan(
                        b2[:], s[:, 0:2], scan_c2[:], s[:, 3:4], op0=mult, op1=add
                    )
                    # tanh(c_new) = tanh(s_f * c_old + i*g)
                    tcv = tp.tile([HID, 1], f32, tag="tc")
                    nc.scalar.activation(
                        tcv[:], state["c"][:], tanh, bias=b2[:, 1:2], scale=s[:, 2:3]
                    )
                    # h = tanh(c_new) * s_o  -> Y
                    nc.vector.tensor_tensor(
                        Y[:, t : t + 1], tcv[:], s[:, 4:5], op=mult
                    )
                    # c_new = s_f*c_old + i*g (off critical path)
                    cnew = zp.tile([HID, 1], f32, tag="c")
                    nc.vector.tensor_scalar(
                        cnew[:], state["c"][:], s[:, 2:3], b2[:, 1:2], op0=mult, op1=add
                    )
                    state["c"] = cnew
                    state["h"] = Y[:, t : t + 1]

        def emit_heads(n):
            zps = hps.tile([NHEAD, chunk], f32, tag="head")
            nc.tensor.matmul(
                zps[:], lhsT=wct_sb[:], rhs=Y[:, ts(n, chunk)], start=True, stop=True
            )
            zst = hsb.tile([NHEAD, chunk], f32, tag="zst")
            nc.vector.tensor_copy(zst[:], zps[:])
            nc.sync.dma_start(Z[:, ts(n, chunk)], zst[:])

        # software-pipelined emission: phase A chunk n+1 overlaps phase B chunk n
        emit_phase_a(0)
        for n in range(nch):
            if n + 1 < nch:
                emit_phase_a(n + 1)
            emit_phase_b(n)
            emit_heads(n)

    nc.compile()
    return nc


def _split_bf16(a):
    import ml_dtypes

    hi = a.astype(ml_dtypes.bfloat16)
    lo = (a - hi.astype(np.float32)).astype(ml_dtypes.bfloat16)
    return np.ascontiguousarray(hi), np.ascontiguousarray(lo)


def _prep_inputs(inputs, mode="bf16"):
    """Host-side folding of masks/biases into weights. Returns per-model maps."""
    import ml_dtypes

    bf = ml_dtypes.bfloat16
    X = np.asarray(inputs["X"], np.float32)[:, 0, :]  # [T, F]
    T = X.shape[0]
    XT = np.ascontiguousarray(X.T)  # [F, T]
    IDT = np.eye(HID, dtype=np.float32)

    W_ih = np.asarray(inputs["W_ih"], np.float32)
    W_hh = np.asarray(inputs["W_hh"], np.float32)
    b_ih = np.asarray(inputs["b_ih"], np.float32)
    b_hh = np.asarray(inputs["b_hh"], np.float32)
    mask_x = np.asarray(inputs["mask_x"], np.float32)
    mask_h = np.asarray(inputs["mask_h"], np.float32)
    heads_w = [np.asarray(inputs[k], np.float32) for k in ("W1", "W2", "W3", "W4")]
    heads_b = [np.asarray(inputs[k], np.float32) for k in ("b1", "b2", "b3", "b4")]

    if mode == "bf16":
        XT_HI, XT_LO = _split_bf16(XT)

    per_model = []
    for r in range(NM):
        wih = (W_ih[r] * mask_x[r][None, :]).copy()
        whh = (W_hh[r] * mask_h[r][None, :]).copy()
        bt = (b_ih[r] + b_hh[r]).copy()
        wih[2 * HID : 3 * HID] *= 2.0
        whh[2 * HID : 3 * HID] *= 2.0
        bt[2 * HID : 3 * HID] *= 2.0
        wc = np.concatenate([w[r] for w in heads_w], axis=0) / NM  # [15, 128]
        if mode == "bf16":
            wih_hi, wih_lo = _split_bf16(np.ascontiguousarray(wih.T))
            bb_hi, bb_lo = _split_bf16(bt[None, :])
            per_model.append(
                {
                    "XT_HI": XT_HI,
                    "XT_LO": XT_LO,
                    "WIH_HI": wih_hi,
                    "WIH_LO": wih_lo,
                    "BB_HI": bb_hi,
                    "BB_LO": bb_lo,
                    "WHH": np.ascontiguousarray(whh.T).astype(bf),
                    "WCT": np.ascontiguousarray(wc.T).astype(bf),
                    "IDT": IDT.astype(bf),
                }
            )
        else:
            per_model.append(
                {
                    "XT": XT,
                    "WIH": np.ascontiguousarray(wih.T),
                    "BB": np.ascontiguousarray(bt[None, :]),
                    "WHH": np.ascontiguousarray(whh.T),
                    "WCT": np.ascontiguousarray(wc.T),
                    "IDT": IDT,
                }
            )
    bias_mean = np.concatenate([b.mean(axis=0) for b in heads_b])  # [15]
    return per_model, bias_mean, T


_CACHE = {}


def _run(inputs, T, mode="bf16", trace=False, n_cores=8):
    from concourse.bass_utils import run_bass_kernel_spmd

    per_model, bias_mean, T_in = _prep_inputs(inputs, mode)
    assert T_in == T
    key = (T, mode)
    if key not in _CACHE:
        _CACHE[key] = _build(T, mode)
    nc = _CACHE[key]
    in_maps = [per_model[min(r, NM - 1)] for r in range(n_cores)]
    res = run_bass_kernel_spmd(nc, in_maps, core_ids=list(range(n_cores)), trace=trace)
    Zsum = np.zeros((NHEAD, T), np.float32)
    for r in range(NM):
        Zsum += res.results[r]["Z"]
    out = Zsum + bias_mean[:, None]
    step_logits = np.ascontiguousarray(out[0:11].T)
    experience = np.ascontiguousarray(out[11:13].T)
    rsd = np.ascontiguousarray(out[13:14].T)
    s = np.ascontiguousarray(out[14:15].T)
    return (step_logits, experience, rsd, s), res


def kernel(**inputs):
    outs, _ = _run(inputs, T_LEN, mode="bf16")
    return outs


# revision 17
# speedup vs baseline: 2.3048x; 1.0470x over previous
"""Trainium2 Bass kernel for the CatRSDNet 5-model MC-dropout LSTM ensemble.

Problem: X [8192,1,1664] -> 5 independent LSTM(1664->128) replicas with
variational dropout masks folded into the weights, 8192 serial timesteps,
then 4 small linear heads ensemble-averaged over the 5 replicas.

Strategy (8 NeuronCores, SPMD, one NEFF):
  - model-parallel: core r simulates ensemble member r (cores 5-7 duplicate
    model 4; their outputs are ignored).
  - Phase A (per core): gx[t] = W_ih_m @ x[t] + b  for all t as large
    matmuls (X transposed on host so F sits on partitions), streamed to a
    DRAM scratch buffer.  fp32 inputs are split into bf16 hi+lo pairs and
    multiplied 3-term (HiHi + HiLo + LoHi) so the PE runs at bf16 rate with
    ~fp24 precision; gx is stored to DRAM as a bf16 hi/lo pair.
  - Phase B: the serial recurrence.  Per step: 4 accumulating 128x128 bf16
    matvecs (W_hh.T stationary, h moving) on top of gx (injected into PSUM
    via identity matmuls, amortized over groups of 4 steps), one Sigmoid
    over all 4 gates (tanh(g) computed as 2*sigmoid(2g)-1 with the g-block
    weights pre-doubled on the host), i*g = (2*s_g-1)*s_i in a single
    tensor_tensor_scan (constant columns baked next to the sigmoid outputs),
    tanh(c_new) in one Tanh with per-partition AP scale (s_f) and bias (i*g)
    applied to c_old, one tensor_tensor for h; the fp32 cell state c_new is
    materialized off the critical path.
  - Heads: one [128,15] matmul per 512-step chunk over the stored Y.
Host folds dropout masks + biases into weights, sums per-core head outputs.
"""

import numpy as np

T_LEN = 8192
FEAT = 1664
HID = 128
NM = 5
G4 = 4 * HID  # 512 gates
NHEAD = 15  # 11 + 2 + 1 + 1
KC = FEAT // 128  # 13 contraction tiles for phase A
GROUP = 4  # steps sharing one PSUM gate tile / one gx-inject matmul


def _build(T=T_LEN, mode="bf16"):
    """Build the Bass program. mode: 'bf16' (fast) or 'fp32' (fallback)."""
    from contextlib import ExitStack

    import concourse.bass as bass
    import concourse.tile as tile
    from concourse import bacc, mybir
    from concourse.bass import ts

    f32 = mybir.dt.float32
    bf16 = mybir.dt.bfloat16
    wdt = bf16 if mode == "bf16" else f32
    chunk = min(512, T)
    nch = T // chunk
    assert T % chunk == 0 and chunk % GROUP == 0

    nc = bacc.Bacc("TRN2", target_bir_lowering=False, debug=False, num_devices=8)
    if mode == "bf16":
        XT_HI = nc.dram_tensor("XT_HI", [FEAT, T], bf16, kind="ExternalInput")
        XT_LO = nc.dram_tensor("XT_LO", [FEAT, T], bf16, kind="ExternalInput")
        WIH_HI = nc.dram_tensor("WIH_HI", [FEAT, G4], bf16, kind="ExternalInput")
        WIH_LO = nc.dram_tensor("WIH_LO", [FEAT, G4], bf16, kind="ExternalInput")
        BB_HI = nc.dram_tensor("BB_HI", [1, G4], bf16, kind="ExternalInput")
        BB_LO = nc.dram_tensor("BB_LO", [1, G4], bf16, kind="ExternalInput")
    else:
        XT = nc.dram_tensor("XT", [FEAT, T], f32, kind="ExternalInput")
        WIH = nc.dram_tensor("WIH", [FEAT, G4], f32, kind="ExternalInput")
        BB = nc.dram_tensor("BB", [1, G4], f32, kind="ExternalInput")
    WHH = nc.dram_tensor("WHH", [HID, G4], wdt, kind="ExternalInput")
    WCT = nc.dram_tensor("WCT", [HID, NHEAD], wdt, kind="ExternalInput")
    IDT = nc.dram_tensor("IDT", [HID, HID], wdt, kind="ExternalInput")
    Z = nc.dram_tensor("Z", [NHEAD, T], f32, kind="ExternalOutput")

    sig = mybir.ActivationFunctionType.Sigmoid
    tanh = mybir.ActivationFunctionType.Tanh
    mult = mybir.AluOpType.mult
    add = mybir.AluOpType.add
    sub = mybir.AluOpType.subtract

    with tile.TileContext(nc) as tc, ExitStack() as ctx:
        const = ctx.enter_context(tc.tile_pool(name="const", bufs=1))
        dram = ctx.enter_context(tc.tile_pool(name="dram", bufs=1, space="DRAM"))
        xtp = ctx.enter_context(tc.tile_pool(name="xtp", bufs=2))
        gxps = ctx.enter_context(tc.tile_pool(name="gxps", bufs=2, space="PSUM"))
        gxsb = ctx.enter_context(tc.tile_pool(name="gxsb", bufs=2))
        hsb = ctx.enter_context(tc.tile_pool(name="hsb", bufs=2))
        gxcp = ctx.enter_context(tc.tile_pool(name="gxcp", bufs=2))
        gpsa = ctx.enter_context(tc.tile_pool(name="gpsa", bufs=2, space="PSUM"))
        gpsb = ctx.enter_context(tc.tile_pool(name="gpsb", bufs=2, space="PSUM"))
        hps = ctx.enter_context(tc.tile_pool(name="hps", bufs=2, space="PSUM"))
        ypool = ctx.enter_context(tc.tile_pool(name="ypool", bufs=1))
        sp = ctx.enter_context(tc.tile_pool(name="sp", bufs=4))
        wp = ctx.enter_context(tc.tile_pool(name="wp", bufs=4))
        zp = ctx.enter_context(tc.tile_pool(name="zp", bufs=4))
        tp = ctx.enter_context(tc.tile_pool(name="tp", bufs=4))

        # ---- constants ----
        whh_sb = const.tile([HID, G4], wdt)
        nc.sync.dma_start(whh_sb[:], WHH[:])
        idt_sb = const.tile([HID, HID], wdt)
        nc.sync.dma_start(idt_sb[:], IDT[:])
        wct_sb = const.tile([HID, NHEAD], wdt)
        nc.sync.dma_start(wct_sb[:], WCT[:])
        if mode == "bf16":
            bbh_sb = const.tile([1, G4], bf16)
            nc.sync.dma_start(bbh_sb[:], BB_HI[:])
            bbl_sb = const.tile([1, G4], bf16)
            nc.sync.dma_start(bbl_sb[:], BB_LO[:])
            wihh_sb = const.tile([128, KC, G4], bf16)
            nc.sync.dma_start(wihh_sb[:], WIH_HI.rearrange("(k p) g -> p k g", p=128)[:])
            wihl_sb = const.tile([128, KC, G4], bf16)
            nc.sync.dma_start(wihl_sb[:], WIH_LO.rearrange("(k p) g -> p k g", p=128)[:])
        else:
            bb_sb = const.tile([1, G4], f32)
            nc.sync.dma_start(bb_sb[:], BB[:])
            wih_sb = const.tile([128, KC, G4], f32)
            nc.sync.dma_start(wih_sb[:], WIH.rearrange("(k p) g -> p k g", p=128)[:])
        ones_sb = const.tile([1, chunk], wdt)
        nc.vector.memset(ones_sb[:], 1.0)
        zero_f32 = const.tile([HID, 1], f32)
        nc.vector.memset(zero_f32[:], 0.0)
        if wdt != f32:
            zero_w = const.tile([HID, 1], wdt)
            nc.vector.memset(zero_w[:], 0.0)
        else:
            zero_w = zero_f32

        if mode == "bf16":
            GXH = dram.tile([4, HID, T], bf16)
            GXL = dram.tile([4, HID, T], bf16)
            gxh_rd = GXH.rearrange("j p t -> p j t")
            gxl_rd = GXL.rearrange("j p t -> p j t")
        else:
            GX = dram.tile([4, HID, T], f32)
            gx_rd = GX.rearrange("j p t -> p j t")

        def emit_phase_a(n):
            if mode == "bf16":
                xth = xtp.tile([128, KC, chunk], bf16, tag="xth")
                nc.sync.dma_start(
                    xth[:],
                    XT_HI.rearrange("(k p) t -> p k t", p=128)[:, :, ts(n, chunk)],
                )
                xtl = xtp.tile([128, KC, chunk], bf16, tag="xtl")
                nc.sync.dma_start(
                    xtl[:],
                    XT_LO.rearrange("(k p) t -> p k t", p=128)[:, :, ts(n, chunk)],
                )
                for j in range(4):
                    ps = gxps.tile([128, chunk], f32, tag="gx")
                    for k in range(KC):
                        for wt, xt_ in (
                            (wihh_sb, xth),
                            (wihh_sb, xtl),
                            (wihl_sb, xth),
                        ):
                            nc.tensor.matmul(
                                ps[:],
                                lhsT=wt[:, k, ts(j, 128)],
                                rhs=xt_[:, k, :],
                                start=(k == 0 and wt is wihh_sb and xt_ is xth),
                                stop=False,
                                skip_group_check=True,
                            )
                    nc.tensor.matmul(
                        ps[:],
                        lhsT=bbh_sb[:, ts(j, 128)],
                        rhs=ones_sb[:],
                        start=False,
                        stop=False,
                        skip_group_check=True,
                    )
                    nc.tensor.matmul(
                        ps[:],
                        lhsT=bbl_sb[:, ts(j, 128)],
                        rhs=ones_sb[:],
                        start=False,
                        stop=True,
                        skip_group_check=True,
                    )
                    sth = gxsb.tile([128, chunk], bf16, tag="sth")
                    nc.vector.tensor_copy(sth[:], ps[:])
                    stw = gxsb.tile([128, chunk], f32, tag="stw")
                    nc.vector.tensor_copy(stw[:], sth[:])
                    stl = gxsb.tile([128, chunk], bf16, tag="stl")
                    nc.vector.tensor_tensor(
                        stl[:], ps[:], stw[:], op=sub
                    )
                    nc.sync.dma_start(GXH[j, :, ts(n, chunk)], sth[:])
                    nc.sync.dma_start(GXL[j, :, ts(n, chunk)], stl[:])
            else:
                xt = xtp.tile([128, KC, chunk], f32, tag="xt")
                nc.sync.dma_start(
                    xt[:], XT.rearrange("(k p) t -> p k t", p=128)[:, :, ts(n, chunk)]
                )
                for j in range(4):
                    ps = gxps.tile([128, chunk], f32, tag="gx")
                    for k in range(KC):
                        nc.tensor.matmul(
                            ps[:],
                            lhsT=wih_sb[:, k, ts(j, 128)],
                            rhs=xt[:, k, :],
                            start=(k == 0),
                            stop=False,
                        )
                    nc.tensor.matmul(
                        ps[:],
                        lhsT=bb_sb[:, ts(j, 128)],
                        rhs=ones_sb[:],
                        start=False,
                        stop=True,
                    )
                    st = gxsb.tile([128, chunk], f32, tag="gxst")
                    nc.vector.tensor_copy(st[:], ps[:])
                    nc.sync.dma_start(GX[j, :, ts(n, chunk)], st[:])

        Y = ypool.tile([HID, T], wdt)
        state = {"c": zero_f32, "h": zero_w}

        # s-tile ring: col0 = constant 2.0, cols 1-4 = sigmoid outputs
        # [i, f, g2, o].  tensor_tensor_scan over cols [0,1] with
        # initial=s_g computes (2*s_g - 1)*s_i = i*g in one DVE op.
        s_ring = []
        for i_ in range(4):
            st_ = sp.tile([HID, 5], f32, name=f"sring{i_}", tag=f"sring{i_}")
            nc.vector.memset(st_[:, 0:1], 2.0)
            s_ring.append(st_)
        scan_c2 = const.tile([HID, 2], f32)
        nc.vector.memset(scan_c2[:, 0:1], -1.0)
        nc.vector.memset(scan_c2[:, 1:2], 0.0)

        def emit_phase_b(n):
            if mode == "bf16":
                gxch = gxcp.tile([128, 4, chunk], bf16, tag="gxch")
                nc.sync.dma_start(gxch[:], gxh_rd[:, :, ts(n, chunk)])
                gxcl = gxcp.tile([128, 4, chunk], bf16, tag="gxcl")
                nc.sync.dma_start(gxcl[:], gxl_rd[:, :, ts(n, chunk)])
                injections = ((gxch, True), (gxcl, False))
            else:
                gxc = gxcp.tile([128, 4, chunk], f32, tag="gxc")
                nc.sync.dma_start(gxc[:], gx_rd[:, :, ts(n, chunk)])
                injections = ((gxc, True),)
            for g0 in range(0, chunk, GROUP):
                # two PSUM banks: A holds (i, g) preacts, B holds (f, o).
                # Bank A's last writer per step is the g-matvec, so the
                # sigmoid over (i, g) can fire two matvecs earlier and the
                # (f, o) sigmoid overlaps the i*g scan on the ACT queue.
                psa = gpsa.tile([128, 2 * GROUP], f32, tag="gatesA")
                psb = gpsb.tile([128, 2 * GROUP], f32, tag="gatesB")
                for src, is_first in injections:
                    for ps_, jsel in ((psa, (0, 2)), (psb, (1, 3))):
                        nc.tensor.matmul(
                            ps_.rearrange("p (t j) -> p j t", j=2)[:],
                            lhsT=idt_sb[:],
                            rhs=src[:, jsel[0] : 4 : 2, g0 : g0 + GROUP],
                            start=is_first,
                            stop=False,
                            skip_group_check=True,
                        )
                for gl in range(GROUP):
                    t = n * chunk + g0 + gl
                    col = 2 * gl
                    h_prev = state["h"]
                    last = gl == GROUP - 1
                    # issue order i, g (bank A) then f, o (bank B)
                    for ps_, j, stop in (
                        (psa, 0, False),
                        (psa, 2, last),
                        (psb, 1, False),
                        (psb, 3, last),
                    ):
                        co = col + (1 if j >= 2 else 0)
                        nc.tensor.matmul(
                            ps_[:, co : co + 1],
                            lhsT=whh_sb[:, ts(j, 128)],
                            rhs=h_prev[:],
                            start=False,
                            stop=stop,
                            skip_group_check=True,
                        )
                    # s layout: [2.0, s_i, s_g, s_f, s_o]
                    s = s_ring[t % 4]
                    nc.scalar.activation(s[:, 1:3], psa[:, col : col + 2], sig)
                    # ig = (2*s_g - 1)*s_i via length-2 scan:
                    #   state=s_g; state=2*state-1; state=s_i*state+0
                    b2 = wp.tile([HID, 2], f32, tag="b2")
                    nc.vector.tensor_tensor_scan(
                        b2[:], s[:, 0:2], scan_c2[:], s[:, 2:3], op0=mult, op1=add
                    )
                    nc.scalar.activation(s[:, 3:5], psb[:, col : col + 2], sig)
                    # tanh(c_new) = tanh(s_f * c_old + i*g)
                    tcv = tp.tile([HID, 1], f32, tag="tc")
                    nc.scalar.activation(
                        tcv[:], state["c"][:], tanh, bias=b2[:, 1:2], scale=s[:, 3:4]
                    )
                    # h = tanh(c_new) * s_o  -> Y
                    nc.vector.tensor_tensor(
                        Y[:, t : t + 1], tcv[:], s[:, 4:5], op=mult
                    )
                    # c_new = s_f*c_old + i*g (off critical path)
                    cnew = zp.tile([HID, 1], f32, tag="c")
                    nc.vector.tensor_scalar(
                        cnew[:], state["c"][:], s[:, 3:4], b2[:, 1:2], op0=mult, op1=add
                    )
                    state["c"] = cnew
                    state["h"] = Y[:, t : t + 1]

        def emit_heads(n):
            zps = hps.tile([NHEAD, chunk], f32, tag="head")
            nc.tensor.matmul(
                zps[:], lhsT=wct_sb[:], rhs=Y[:, ts(n, chunk)], start=True, stop=True
            )
            zst = hsb.tile([NHEAD, chunk], f32, tag="zst")
            nc.vector.tensor_copy(zst[:], zps[:])
            nc.sync.dma_start(Z[:, ts(n, chunk)], zst[:])

        # software-pipelined emission: phase A chunk n+1 overlaps phase B chunk n
        emit_phase_a(0)
        for n in range(nch):
            if n + 1 < nch:
                emit_phase_a(n + 1)
            emit_phase_b(n)
            emit_heads(n)

    nc.compile()
    return nc


def _split_bf16(a):
    import ml_dtypes

    hi = a.astype(ml_dtypes.bfloat16)
    lo = (a - hi.astype(np.float32)).astype(ml_dtypes.bfloat16)
    return np.ascontiguousarray(hi), np.ascontiguousarray(lo)


def _prep_inputs(inputs, mode="bf16"):
    """Host-side folding of masks/biases into weights. Returns per-model maps."""
    import ml_dtypes

    bf = ml_dtypes.bfloat16
    X = np.asarray(inputs["X"], np.float32)[:, 0, :]  # [T, F]
    T = X.shape[0]
    XT = np.ascontiguousarray(X.T)  # [F, T]
    IDT = np.eye(HID, dtype=np.float32)

    W_ih = np.asarray(inputs["W_ih"], np.float32)
    W_hh = np.asarray(inputs["W_hh"], np.float32)
    b_ih = np.asarray(inputs["b_ih"], np.float32)
    b_hh = np.asarray(inputs["b_hh"], np.float32)
    mask_x = np.asarray(inputs["mask_x"], np.float32)
    mask_h = np.asarray(inputs["mask_h"], np.float32)
    heads_w = [np.asarray(inputs[k], np.float32) for k in ("W1", "W2", "W3", "W4")]
    heads_b = [np.asarray(inputs[k], np.float32) for k in ("b1", "b2", "b3", "b4")]

    if mode == "bf16":
        XT_HI, XT_LO = _split_bf16(XT)

    per_model = []
    for r in range(NM):
        wih = (W_ih[r] * mask_x[r][None, :]).copy()
        whh = (W_hh[r] * mask_h[r][None, :]).copy()
        bt = (b_ih[r] + b_hh[r]).copy()
        wih[2 * HID : 3 * HID] *= 2.0
        whh[2 * HID : 3 * HID] *= 2.0
        bt[2 * HID : 3 * HID] *= 2.0
        wc = np.concatenate([w[r] for w in heads_w], axis=0) / NM  # [15, 128]
        if mode == "bf16":
            wih_hi, wih_lo = _split_bf16(np.ascontiguousarray(wih.T))
            bb_hi, bb_lo = _split_bf16(bt[None, :])
            per_model.append(
                {
                    "XT_HI": XT_HI,
                    "XT_LO": XT_LO,
                    "WIH_HI": wih_hi,
                    "WIH_LO": wih_lo,
                    "BB_HI": bb_hi,
                    "BB_LO": bb_lo,
                    "WHH": np.ascontiguousarray(whh.T).astype(bf),
                    "WCT": np.ascontiguousarray(wc.T).astype(bf),
                    "IDT": IDT.astype(bf),
                }
            )
        else:
            per_model.append(
                {
                    "XT": XT,
                    "WIH": np.ascontiguousarray(wih.T),
                    "BB": np.ascontiguousarray(bt[None, :]),
                    "WHH": np.ascontiguousarray(whh.T),
                    "WCT": np.ascontiguousarray(wc.T),
                    "IDT": IDT,
                }
            )
    bias_mean = np.concatenate([b.mean(axis=0) for b in heads_b])  # [15]
    return per_model, bias_mean, T


_CACHE = {}


def _run(inputs, T, mode="bf16", trace=False, n_cores=8):
    from concourse.bass_utils import run_bass_kernel_spmd

    per_model, bias_mean, T_in = _prep_inputs(inputs, mode)
    assert T_in == T
    key = (T, mode)
    if key not in _CACHE:
        _CACHE[key] = _build(T, mode)
    nc = _CACHE[key]
    in_maps = [per_model[min(r, NM - 1)] for r in range(n_cores)]
    res = run_bass_kernel_spmd(nc, in_maps, core_ids=list(range(n_cores)), trace=trace)
    Zsum = np.zeros((NHEAD, T), np.float32)
    for r in range(NM):
        Zsum += res.results[r]["Z"]
    out = Zsum + bias_mean[:, None]
    step_logits = np.ascontiguousarray(out[0:11].T)
    experience = np.ascontiguousarray(out[11:13].T)
    rsd = np.ascontiguousarray(out[13:14].T)
    s = np.ascontiguousarray(out[14:15].T)
    return (step_logits, experience, rsd, s), res


def kernel(**inputs):
    outs, _ = _run(inputs, T_LEN, mode="bf16")
    return outs


# revision 18
# speedup vs baseline: 2.3131x; 1.0036x over previous
"""Trainium2 Bass kernel for the CatRSDNet 5-model MC-dropout LSTM ensemble.

Problem: X [8192,1,1664] -> 5 independent LSTM(1664->128) replicas with
variational dropout masks folded into the weights, 8192 serial timesteps,
then 4 small linear heads ensemble-averaged over the 5 replicas.

Strategy (8 NeuronCores, SPMD, one NEFF):
  - model-parallel: core r simulates ensemble member r (cores 5-7 duplicate
    model 4; their outputs are ignored).
  - Phase A (per core): gx[t] = W_ih_m @ x[t] + b  for all t as large
    matmuls (X transposed on host so F sits on partitions), streamed to a
    DRAM scratch buffer.  fp32 inputs are split into bf16 hi+lo pairs and
    multiplied 3-term (HiHi + HiLo + LoHi) so the PE runs at bf16 rate with
    ~fp24 precision; gx is stored to DRAM as a bf16 hi/lo pair.
  - Phase B: the serial recurrence.  Per step: 4 accumulating 128x128 bf16
    matvecs (W_hh.T stationary, h moving) on top of gx (injected into PSUM
    via identity matmuls, amortized over groups of 4 steps), one Sigmoid
    over all 4 gates (tanh(g) computed as 2*sigmoid(2g)-1 with the g-block
    weights pre-doubled on the host), i*g = (2*s_g-1)*s_i in a single
    tensor_tensor_scan (constant columns baked next to the sigmoid outputs),
    tanh(c_new) in one Tanh with per-partition AP scale (s_f) and bias (i*g)
    applied to c_old, one tensor_tensor for h; the fp32 cell state c_new is
    materialized off the critical path.
  - Heads: one [128,15] matmul per 512-step chunk over the stored Y.
Host folds dropout masks + biases into weights, sums per-core head outputs.
"""

import numpy as np

T_LEN = 8192
FEAT = 1664
HID = 128
NM = 5
G4 = 4 * HID  # 512 gates
NHEAD = 15  # 11 + 2 + 1 + 1
KC = FEAT // 128  # 13 contraction tiles for phase A
GROUP = 4  # steps sharing one PSUM gate tile / one gx-inject matmul


def _build(T=T_LEN, mode="bf16"):
    """Build the Bass program. mode: 'bf16' (fast) or 'fp32' (fallback)."""
    from contextlib import ExitStack

    import concourse.bass as bass
    import concourse.tile as tile
    from concourse import bacc, mybir
    from concourse.bass import ts

    f32 = mybir.dt.float32
    bf16 = mybir.dt.bfloat16
    wdt = bf16 if mode == "bf16" else f32
    chunk = min(512, T)
    assert T % chunk == 0 and chunk % GROUP == 0
    # split the first 512 steps into 64+448 so the recurrence starts
    # ~60us earlier (phase A's first chunk is on the critical path)
    if chunk == 512:
        chs = [(0, 64), (64, 448)] + [(c, 512) for c in range(512, T, 512)]
    else:
        chs = [(0, chunk)]

    nc = bacc.Bacc("TRN2", target_bir_lowering=False, debug=False, num_devices=8)
    if mode == "bf16":
        XT_HI = nc.dram_tensor("XT_HI", [FEAT, T], bf16, kind="ExternalInput")
        XT_LO = nc.dram_tensor("XT_LO", [FEAT, T], bf16, kind="ExternalInput")
        WIH_HI = nc.dram_tensor("WIH_HI", [FEAT, G4], bf16, kind="ExternalInput")
        WIH_LO = nc.dram_tensor("WIH_LO", [FEAT, G4], bf16, kind="ExternalInput")
        BB_HI = nc.dram_tensor("BB_HI", [1, G4], bf16, kind="ExternalInput")
        BB_LO = nc.dram_tensor("BB_LO", [1, G4], bf16, kind="ExternalInput")
    else:
        XT = nc.dram_tensor("XT", [FEAT, T], f32, kind="ExternalInput")
        WIH = nc.dram_tensor("WIH", [FEAT, G4], f32, kind="ExternalInput")
        BB = nc.dram_tensor("BB", [1, G4], f32, kind="ExternalInput")
    WHH = nc.dram_tensor("WHH", [HID, G4], wdt, kind="ExternalInput")
    WCT = nc.dram_tensor("WCT", [HID, NHEAD], wdt, kind="ExternalInput")
    IDT = nc.dram_tensor("IDT", [HID, HID], wdt, kind="ExternalInput")
    Z = nc.dram_tensor("Z", [NHEAD, T], f32, kind="ExternalOutput")

    sig = mybir.ActivationFunctionType.Sigmoid
    tanh = mybir.ActivationFunctionType.Tanh
    mult = mybir.AluOpType.mult
    add = mybir.AluOpType.add
    sub = mybir.AluOpType.subtract

    with tile.TileContext(nc) as tc, ExitStack() as ctx:
        const = ctx.enter_context(tc.tile_pool(name="const", bufs=1))
        dram = ctx.enter_context(tc.tile_pool(name="dram", bufs=1, space="DRAM"))
        xtp = ctx.enter_context(tc.tile_pool(name="xtp", bufs=2))
        gxps = ctx.enter_context(tc.tile_pool(name="gxps", bufs=2, space="PSUM"))
        gxsb = ctx.enter_context(tc.tile_pool(name="gxsb", bufs=2))
        hsb = ctx.enter_context(tc.tile_pool(name="hsb", bufs=2))
        gxcp = ctx.enter_context(tc.tile_pool(name="gxcp", bufs=2))
        gpsa = ctx.enter_context(tc.tile_pool(name="gpsa", bufs=2, space="PSUM"))
        gpsb = ctx.enter_context(tc.tile_pool(name="gpsb", bufs=2, space="PSUM"))
        hps = ctx.enter_context(tc.tile_pool(name="hps", bufs=2, space="PSUM"))
        ypool = ctx.enter_context(tc.tile_pool(name="ypool", bufs=1))
        sp = ctx.enter_context(tc.tile_pool(name="sp", bufs=4))
        wp = ctx.enter_context(tc.tile_pool(name="wp", bufs=4))
        zp = ctx.enter_context(tc.tile_pool(name="zp", bufs=4))
        tp = ctx.enter_context(tc.tile_pool(name="tp", bufs=4))

        # ---- constants ----
        whh_sb = const.tile([HID, G4], wdt)
        nc.sync.dma_start(whh_sb[:], WHH[:])
        idt_sb = const.tile([HID, HID], wdt)
        nc.sync.dma_start(idt_sb[:], IDT[:])
        wct_sb = const.tile([HID, NHEAD], wdt)
        nc.sync.dma_start(wct_sb[:], WCT[:])
        if mode == "bf16":
            bbh_sb = const.tile([1, G4], bf16)
            nc.sync.dma_start(bbh_sb[:], BB_HI[:])
            bbl_sb = const.tile([1, G4], bf16)
            nc.sync.dma_start(bbl_sb[:], BB_LO[:])
            wihh_sb = const.tile([128, KC, G4], bf16)
            nc.sync.dma_start(wihh_sb[:], WIH_HI.rearrange("(k p) g -> p k g", p=128)[:])
            wihl_sb = const.tile([128, KC, G4], bf16)
            nc.sync.dma_start(wihl_sb[:], WIH_LO.rearrange("(k p) g -> p k g", p=128)[:])
        else:
            bb_sb = const.tile([1, G4], f32)
            nc.sync.dma_start(bb_sb[:], BB[:])
            wih_sb = const.tile([128, KC, G4], f32)
            nc.sync.dma_start(wih_sb[:], WIH.rearrange("(k p) g -> p k g", p=128)[:])
        ones_sb = const.tile([1, chunk], wdt)
        nc.vector.memset(ones_sb[:], 1.0)
        zero_f32 = const.tile([HID, 1], f32)
        nc.vector.memset(zero_f32[:], 0.0)
        if wdt != f32:
            zero_w = const.tile([HID, 1], wdt)
            nc.vector.memset(zero_w[:], 0.0)
        else:
            zero_w = zero_f32

        if mode == "bf16":
            GXH = dram.tile([4, HID, T], bf16)
            GXL = dram.tile([4, HID, T], bf16)
            gxh_rd = GXH.rearrange("j p t -> p j t")
            gxl_rd = GXL.rearrange("j p t -> p j t")
        else:
            GX = dram.tile([4, HID, T], f32)
            gx_rd = GX.rearrange("j p t -> p j t")

        def emit_phase_a(c0, sz):
            if mode == "bf16":
                xth = xtp.tile([128, KC, sz], bf16, tag="xth", padded_shape=[128, KC, chunk])
                nc.sync.dma_start(
                    xth[:],
                    XT_HI.rearrange("(k p) t -> p k t", p=128)[:, :, c0 : c0 + sz],
                )
                xtl = xtp.tile([128, KC, sz], bf16, tag="xtl", padded_shape=[128, KC, chunk])
                nc.sync.dma_start(
                    xtl[:],
                    XT_LO.rearrange("(k p) t -> p k t", p=128)[:, :, c0 : c0 + sz],
                )
                for j in range(4):
                    ps = gxps.tile([128, sz], f32, tag="gx", padded_shape=[128, chunk])
                    for k in range(KC):
                        for wt, xt_ in (
                            (wihh_sb, xth),
                            (wihh_sb, xtl),
                            (wihl_sb, xth),
                        ):
                            nc.tensor.matmul(
                                ps[:],
                                lhsT=wt[:, k, ts(j, 128)],
                                rhs=xt_[:, k, :],
                                start=(k == 0 and wt is wihh_sb and xt_ is xth),
                                stop=False,
                                skip_group_check=True,
                            )
                    nc.tensor.matmul(
                        ps[:],
                        lhsT=bbh_sb[:, ts(j, 128)],
                        rhs=ones_sb[:, :sz],
                        start=False,
                        stop=False,
                        skip_group_check=True,
                    )
                    nc.tensor.matmul(
                        ps[:],
                        lhsT=bbl_sb[:, ts(j, 128)],
                        rhs=ones_sb[:, :sz],
                        start=False,
                        stop=True,
                        skip_group_check=True,
                    )
                    sth = gxsb.tile([128, sz], bf16, tag="sth", padded_shape=[128, chunk])
                    nc.vector.tensor_copy(sth[:], ps[:])
                    stw = gxsb.tile([128, sz], f32, tag="stw", padded_shape=[128, chunk])
                    nc.vector.tensor_copy(stw[:], sth[:])
                    stl = gxsb.tile([128, sz], bf16, tag="stl", padded_shape=[128, chunk])
                    nc.vector.tensor_tensor(
                        stl[:], ps[:], stw[:], op=sub
                    )
                    nc.sync.dma_start(GXH[j, :, c0 : c0 + sz], sth[:])
                    nc.sync.dma_start(GXL[j, :, c0 : c0 + sz], stl[:])
            else:
                xt = xtp.tile([128, KC, sz], f32, tag="xt", padded_shape=[128, KC, chunk])
                nc.sync.dma_start(
                    xt[:], XT.rearrange("(k p) t -> p k t", p=128)[:, :, c0 : c0 + sz]
                )
                for j in range(4):
                    ps = gxps.tile([128, sz], f32, tag="gx", padded_shape=[128, chunk])
                    for k in range(KC):
                        nc.tensor.matmul(
                            ps[:],
                            lhsT=wih_sb[:, k, ts(j, 128)],
                            rhs=xt[:, k, :],
                            start=(k == 0),
                            stop=False,
                        )
                    nc.tensor.matmul(
                        ps[:],
                        lhsT=bb_sb[:, ts(j, 128)],
                        rhs=ones_sb[:, :sz],
                        start=False,
                        stop=True,
                    )
                    st = gxsb.tile([128, sz], f32, tag="gxst", padded_shape=[128, chunk])
                    nc.vector.tensor_copy(st[:], ps[:])
                    nc.sync.dma_start(GX[j, :, c0 : c0 + sz], st[:])

        Y = ypool.tile([HID, T], wdt)
        state = {"c": zero_f32, "h": zero_w}

        # s-tile ring: col0 = constant 2.0, cols 1-4 = sigmoid outputs
        # [i, f, g2, o].  tensor_tensor_scan over cols [0,1] with
        # initial=s_g computes (2*s_g - 1)*s_i = i*g in one DVE op.
        s_ring = []
        for i_ in range(4):
            st_ = sp.tile([HID, 5], f32, name=f"sring{i_}", tag=f"sring{i_}")
            nc.vector.memset(st_[:, 0:1], 2.0)
            s_ring.append(st_)
        scan_c2 = const.tile([HID, 2], f32)
        nc.vector.memset(scan_c2[:, 0:1], -1.0)
        nc.vector.memset(scan_c2[:, 1:2], 0.0)

        def emit_phase_b(c0, sz):
            if mode == "bf16":
                gxch = gxcp.tile([128, 4, sz], bf16, tag="gxch", padded_shape=[128, 4, chunk])
                nc.sync.dma_start(gxch[:], gxh_rd[:, :, c0 : c0 + sz])
                gxcl = gxcp.tile([128, 4, sz], bf16, tag="gxcl", padded_shape=[128, 4, chunk])
                nc.sync.dma_start(gxcl[:], gxl_rd[:, :, c0 : c0 + sz])
                injections = ((gxch, True), (gxcl, False))
            else:
                gxc = gxcp.tile([128, 4, sz], f32, tag="gxc", padded_shape=[128, 4, chunk])
                nc.sync.dma_start(gxc[:], gx_rd[:, :, c0 : c0 + sz])
                injections = ((gxc, True),)
            for g0 in range(0, sz, GROUP):
                # two PSUM banks: A holds (i, g) preacts, B holds (f, o).
                # Bank A's last writer per step is the g-matvec, so the
                # sigmoid over (i, g) can fire two matvecs earlier and the
                # (f, o) sigmoid overlaps the i*g scan on the ACT queue.
                psa = gpsa.tile([128, 2 * GROUP], f32, tag="gatesA")
                psb = gpsb.tile([128, 2 * GROUP], f32, tag="gatesB")
                for src, is_first in injections:
                    for ps_, jsel in ((psa, (0, 2)), (psb, (1, 3))):
                        nc.tensor.matmul(
                            ps_.rearrange("p (t j) -> p j t", j=2)[:],
                            lhsT=idt_sb[:],
                            rhs=src[:, jsel[0] : 4 : 2, g0 : g0 + GROUP],
                            start=is_first,
                            stop=False,
                            skip_group_check=True,
                        )
                for gl in range(GROUP):
                    t = c0 + g0 + gl
                    col = 2 * gl
                    h_prev = state["h"]
                    last = gl == GROUP - 1
                    # issue order i, g (bank A) then f, o (bank B)
                    for ps_, j, stop in (
                        (psa, 0, False),
                        (psa, 2, last),
                        (psb, 1, False),
                        (psb, 3, last),
                    ):
                        co = col + (1 if j >= 2 else 0)
                        nc.tensor.matmul(
                            ps_[:, co : co + 1],
                            lhsT=whh_sb[:, ts(j, 128)],
                            rhs=h_prev[:],
                            start=False,
                            stop=stop,
                            skip_group_check=True,
                        )
                    # s layout: [2.0, s_i, s_g, s_f, s_o]
                    s = s_ring[t % 4]
                    nc.scalar.activation(s[:, 1:3], psa[:, col : col + 2], sig)
                    # ig = (2*s_g - 1)*s_i via length-2 scan:
                    #   state=s_g; state=2*state-1; state=s_i*state+0
                    b2 = wp.tile([HID, 2], f32, tag="b2")
                    nc.vector.tensor_tensor_scan(
                        b2[:], s[:, 0:2], scan_c2[:], s[:, 2:3], op0=mult, op1=add
                    )
                    nc.scalar.activation(s[:, 3:5], psb[:, col : col + 2], sig)
                    # tanh(c_new) = tanh(s_f * c_old + i*g)
                    tcv = tp.tile([HID, 1], f32, tag="tc")
                    nc.scalar.activation(
                        tcv[:], state["c"][:], tanh, bias=b2[:, 1:2], scale=s[:, 3:4]
                    )
                    # h = tanh(c_new) * s_o  -> Y
                    nc.vector.tensor_tensor(
                        Y[:, t : t + 1], tcv[:], s[:, 4:5], op=mult
                    )
                    # c_new = s_f*c_old + i*g (off critical path)
                    cnew = zp.tile([HID, 1], f32, tag="c")
                    nc.vector.tensor_scalar(
                        cnew[:], state["c"][:], s[:, 3:4], b2[:, 1:2], op0=mult, op1=add
                    )
                    state["c"] = cnew
                    state["h"] = Y[:, t : t + 1]

        def emit_heads(c0, sz):
            zps = hps.tile([NHEAD, sz], f32, tag="head", padded_shape=[NHEAD, chunk])
            nc.tensor.matmul(
                zps[:], lhsT=wct_sb[:], rhs=Y[:, c0 : c0 + sz], start=True, stop=True
            )
            zst = hsb.tile([NHEAD, sz], f32, tag="zst", padded_shape=[NHEAD, chunk])
            nc.vector.tensor_copy(zst[:], zps[:])
            nc.sync.dma_start(Z[:, c0 : c0 + sz], zst[:])

        # software-pipelined emission: phase A chunk n+1 overlaps phase B chunk n
        emit_phase_a(*chs[0])
        for n in range(len(chs)):
            if n + 1 < len(chs):
                emit_phase_a(*chs[n + 1])
            emit_phase_b(*chs[n])
            emit_heads(*chs[n])

    nc.compile()
    return nc


def _split_bf16(a):
    import ml_dtypes

    hi = a.astype(ml_dtypes.bfloat16)
    lo = (a - hi.astype(np.float32)).astype(ml_dtypes.bfloat16)
    return np.ascontiguousarray(hi), np.ascontiguousarray(lo)


def _prep_inputs(inputs, mode="bf16"):
    """Host-side folding of masks/biases into weights. Returns per-model maps."""
    import ml_dtypes

    bf = ml_dtypes.bfloat16
    X = np.asarray(inputs["X"], np.float32)[:, 0, :]  # [T, F]
    T = X.shape[0]
    XT = np.ascontiguousarray(X.T)  # [F, T]
    IDT = np.eye(HID, dtype=np.float32)

    W_ih = np.asarray(inputs["W_ih"], np.float32)
    W_hh = np.asarray(inputs["W_hh"], np.float32)
    b_ih = np.asarray(inputs["b_ih"], np.float32)
    b_hh = np.asarray(inputs["b_hh"], np.float32)
    mask_x = np.asarray(inputs["mask_x"], np.float32)
    mask_h = np.asarray(inputs["mask_h"], np.float32)
    heads_w = [np.asarray(inputs[k], np.float32) for k in ("W1", "W2", "W3", "W4")]
    heads_b = [np.asarray(inputs[k], np.float32) for k in ("b1", "b2", "b3", "b4")]

    if mode == "bf16":
        XT_HI, XT_LO = _split_bf16(XT)

    per_model = []
    for r in range(NM):
        wih = (W_ih[r] * mask_x[r][None, :]).copy()
        whh = (W_hh[r] * mask_h[r][None, :]).copy()
        bt = (b_ih[r] + b_hh[r]).copy()
        wih[2 * HID : 3 * HID] *= 2.0
        whh[2 * HID : 3 * HID] *= 2.0
        bt[2 * HID : 3 * HID] *= 2.0
        wc = np.concatenate([w[r] for w in heads_w], axis=0) / NM  # [15, 128]
        if mode == "bf16":
            wih_hi, wih_lo = _split_bf16(np.ascontiguousarray(wih.T))
            bb_hi, bb_lo = _split_bf16(bt[None, :])
            per_model.append(
                {
                    "XT_HI": XT_HI,
                    "XT_LO": XT_LO,
                    "WIH_HI": wih_hi,
                    "WIH_LO": wih_lo,
                    "BB_HI": bb_hi,
                    "BB_LO": bb_lo,
                    "WHH": np.ascontiguousarray(whh.T).astype(bf),
                    "WCT": np.ascontiguousarray(wc.T).astype(bf),
                    "IDT": IDT.astype(bf),
                }
            )
        else:
            per_model.append(
                {
                    "XT": XT,
                    "WIH": np.ascontiguousarray(wih.T),
                    "BB": np.ascontiguousarray(bt[None, :]),
                    "WHH": np.ascontiguousarray(whh.T),
                    "WCT": np.ascontiguousarray(wc.T),
                    "IDT": IDT,
                }
            )
    bias_mean = np.concatenate([b.mean(axis=0) for b in heads_b])  # [15]
    return per_model, bias_mean, T


_CACHE = {}


def _run(inputs, T, mode="bf16", trace=False, n_cores=8):
    from concourse.bass_utils import run_bass_kernel_spmd

    per_model, bias_mean, T_in = _prep_inputs(inputs, mode)
    assert T_in == T
    key = (T, mode)
    if key not in _CACHE:
        _CACHE[key] = _build(T, mode)
    nc = _CACHE[key]
    in_maps = [per_model[min(r, NM - 1)] for r in range(n_cores)]
    res = run_bass_kernel_spmd(nc, in_maps, core_ids=list(range(n_cores)), trace=trace)
    Zsum = np.zeros((NHEAD, T), np.float32)
    for r in range(NM):
        Zsum += res.results[r]["Z"]
    out = Zsum + bias_mean[:, None]
    step_logits = np.ascontiguousarray(out[0:11].T)
    experience = np.ascontiguousarray(out[11:13].T)
    rsd = np.ascontiguousarray(out[13:14].T)
    s = np.ascontiguousarray(out[14:15].T)
    return (step_logits, experience, rsd, s), res


def kernel(**inputs):
    outs, _ = _run(inputs, T_LEN, mode="bf16")
    return outs
